# revision 9
# baseline (speedup 1.0000x reference)
"""GCN autoencoder (2-layer GCNConv encoder + inner-product decoder) on
8 Trainium2 NeuronCores.

Strategy (graph/data parallel, per the node-range sharding):
  - Nodes are sharded 2048/core by TARGET (col) range; edges are bucketed
    host-side by target tile (128 targets) and padded to a static block
    count so all 8 cores run one shared NEFF.
  - deg/dinv are computed on-device from a host-laid-out padded-CSR of
    edge weights (placement is host index work; all value math on device).
  - GCNConv is decomposed as out[c] = dinv[c] * sum_e w_e * G[row_e] + b
    with G[r] = dinv[r] * (x W)[r], so no per-edge dinv gathers are needed.
  - The scatter-add is a dense one-hot matmul: for each block of 128 edges
    (all targeting one 128-target tile), build onehot[e,t] = w_e*(col_e==t)
    with iota + tensor_scalar(is_equal, mult), gather the 128 source rows
    of G with one bulk dma_gather, and accumulate PSUM[t,f] += onehot.T @ G.
  - Full G is assembled between layers with an AllGather collective.
  - Decoder: zT ([64,16384]) lives in SBUF; od rows are sharded per core;
    softplus is Ln(1+Exp(x)) (both funcs in one ACT table).
"""

import os
import numpy as np

import concourse.bass as bass
import concourse.tile as tile
from concourse import bacc, mybir
from concourse.bass_utils import run_bass_kernel_spmd
from concourse.masks import make_identity
from contextlib import ExitStack

P = 128
NCORES = 8
N = 16384
IN_DIM = 256
HID = 128
EMB = 64
E = 524288
NLOC = N // NCORES            # 2048 nodes per core
NT = NLOC // P                # 16 target tiles per core
TILES_PER_GATHER = 2          # gather granularity (tiles)

FP = mybir.dt.float32
I16 = mybir.dt.int16

_AF = mybir.ActivationFunctionType
_OP = mybir.AluOpType


# ----------------------------------------------------------------------
# host-side layout prep (index work only; all value math runs on device)
# ----------------------------------------------------------------------

def _prep_inputs(x, row, col, w):
    """Bucket edges by target tile, pad to a static block count, and build
    the per-core input arrays."""
    rows_all = np.concatenate([row, np.arange(N, dtype=np.int64)])
    cols_all = np.concatenate([col, np.arange(N, dtype=np.int64)])
    w_all = np.concatenate([w, np.ones(N, np.float32)]).astype(np.float32)

    tile_id = (cols_all // P).astype(np.int64)          # 0..127 global tiles
    order = np.argsort(tile_id, kind="stable")
    rows_s, cols_s, w_s = rows_all[order], cols_all[order], w_all[order]
    tile_s = tile_id[order]
    counts = np.bincount(tile_s, minlength=N // P)       # edges per tile
    NBT = int(np.ceil(counts.max() / P))                 # blocks per tile
    starts = np.concatenate([[0], np.cumsum(counts)])

    # per-target degree CSR width
    deg_counts = np.bincount(cols_all, minlength=N)
    DMAX = int(deg_counts.max())

    per_core = []
    NB_TOT = NT * NBT
    NI = NB_TOT * P
    for c in range(NCORES):
        idx_flat = np.zeros(NI, np.int64)
        colv = np.zeros((P, NB_TOT), np.float32)
        wv = np.zeros((P, NB_TOT), np.float32)
        wcsr = np.zeros((P, NT, DMAX), np.float32)
        for lt in range(NT):
            t = c * NT + lt
            s, e = starts[t], starts[t + 1]
            cnt = e - s
            base = lt * NBT * P
            idx_flat[base:base + cnt] = rows_s[s:e]
            cl = (cols_s[s:e] - t * P).astype(np.float32)
            ww = w_s[s:e]
            flat_cl = np.zeros(NBT * P, np.float32)
            flat_w = np.zeros(NBT * P, np.float32)
            flat_cl[:cnt] = cl
            flat_w[:cnt] = ww
            colv[:, lt * NBT:(lt + 1) * NBT] = flat_cl.reshape(NBT, P).T
            wv[:, lt * NBT:(lt + 1) * NBT] = flat_w.reshape(NBT, P).T
            # padded-CSR of weights per target for the degree reduce
            tgt_local = (cols_s[s:e] - t * P).astype(np.int64)
            slot = np.zeros(P, np.int64)
            for j in range(cnt):
                p = tgt_local[j]
                wcsr[p, lt, slot[p]] = ww[j]
                slot[p] += 1
        # dma_gather idx layout: unwrapped[i] = idxs[i%16, i//16],
        # replicated across the 8 groups of 16 partitions
        idx16 = np.zeros((16, NI // 16), np.int16)
        idx16[np.arange(NI) % 16, np.arange(NI) // 16] = idx_flat.astype(np.int16)
        idx_lay = np.tile(idx16, (8, 1))
        per_core.append(dict(idx=idx_lay, colv=colv, wv=wv,
                             wcsr=wcsr.reshape(P, NT * DMAX)))
    return per_core, NBT, DMAX


# ----------------------------------------------------------------------
# device kernel
# ----------------------------------------------------------------------

def _build(NBT, DMAX):
    NB_TOT = NT * NBT
    NI = NB_TOT * P
    GB = TILES_PER_GATHER * NBT          # blocks per gather
    NIG = GB * P                         # idxs per gather
    NGATHER = NT // TILES_PER_GATHER

    nc = bacc.Bacc("TRN2", target_bir_lowering=False, debug=False,
                   num_devices=NCORES)

    xT_in = nc.dram_tensor("xT_in", [IN_DIM, NLOC], FP, kind="ExternalInput").ap()
    w1_in = nc.dram_tensor("w1_in", [IN_DIM, HID], FP, kind="ExternalInput").ap()
    b1_in = nc.dram_tensor("b1_in", [1, HID], FP, kind="ExternalInput").ap()
    w2_in = nc.dram_tensor("w2_in", [HID, EMB], FP, kind="ExternalInput").ap()
    b2_in = nc.dram_tensor("b2_in", [EMB, 1], FP, kind="ExternalInput").ap()
    db_in = nc.dram_tensor("db_in", [1, 1], FP, kind="ExternalInput").ap()
    idx_in = nc.dram_tensor("idx_in", [P, NI // 16], I16, kind="ExternalInput").ap()
    col_in = nc.dram_tensor("col_in", [P, NB_TOT], FP, kind="ExternalInput").ap()
    wv_in = nc.dram_tensor("wv_in", [P, NB_TOT], FP, kind="ExternalInput").ap()
    wcsr_in = nc.dram_tensor("wcsr_in", [P, NT * DMAX], FP, kind="ExternalInput").ap()

    od_out = nc.dram_tensor("od_out", [NLOC, N], FP, kind="ExternalOutput").ap()
    z_out = nc.dram_tensor("z_out", [EMB, NLOC], FP, kind="ExternalOutput").ap()

    with tile.TileContext(nc) as tc, ExitStack() as ctx:
        dram = ctx.enter_context(tc.tile_pool(name="dram", bufs=1, space="DRAM"))
        g1loc = dram.tile([NLOC, HID], FP)
        g1full = dram.tile([N, HID], FP, addr_space="Shared")
        g2loc = dram.tile([NLOC, HID], FP)
        g2full = dram.tile([N, HID], FP, addr_space="Shared")
        ztloc = dram.tile([EMB, NLOC], FP)
        ztstack = dram.tile([NCORES * EMB, NLOC], FP, addr_space="Shared")

        # ---- persistent small tiles ---------------------------------
        const = ctx.enter_context(tc.tile_pool(name="const", bufs=1))
        iota = const.tile([P, P], FP)
        nc.gpsimd.iota(iota[:], pattern=[[1, P]], base=0, channel_multiplier=0,
                       allow_small_or_imprecise_dtypes=True)
        ident = const.tile([P, P], FP)
        make_identity(nc, ident[:])
        ones1 = const.tile([1, P], FP)
        nc.vector.memset(ones1[:], 1.0)

        w1sb = const.tile([P, 2, HID], FP)   # [k-half][kp, f1]
        nc.sync.dma_start(w1sb[:, 0, :], w1_in[0:P, :])
        nc.sync.dma_start(w1sb[:, 1, :], w1_in[P:IN_DIM, :])
        w2sb = const.tile([HID, EMB], FP)
        nc.sync.dma_start(w2sb[:], w2_in[:])
        b1sb = const.tile([1, HID], FP)
        nc.sync.dma_start(b1sb[:], b1_in[:])
        b2sb = const.tile([EMB, 1], FP)
        nc.sync.dma_start(b2sb[:], b2_in[:])
        dbsb = const.tile([1, 1], FP)
        nc.sync.dma_start(dbsb[:], db_in[:])

        colv = const.tile([P, NB_TOT], FP)
        nc.sync.dma_start(colv[:], col_in[:])
        wv = const.tile([P, NB_TOT], FP)
        nc.sync.dma_start(wv[:], wv_in[:])
        idxs = const.tile([P, NI // 16], I16)
        nc.sync.dma_start(idxs[:], idx_in[:])

        # ---- phase 1: degree -> dinv --------------------------------
        with tc.tile_pool(name="p1", bufs=1) as p1, \
             tc.tile_pool(name="p1ps", bufs=1, space="PSUM") as p1ps:
            wcsr = p1.tile([P, NT, DMAX], FP)
            nc.sync.dma_start(wcsr[:, :, :], wcsr_in[:].rearrange(
                "p (t d) -> p t d", t=NT))
            deg = const.tile([P, NT], FP)
            nc.vector.tensor_reduce(deg[:], wcsr[:, :, :],
                                    axis=mybir.AxisListType.X, op=_OP.add)
            sdeg = const.tile([P, NT], FP)
            nc.scalar.activation(sdeg[:], deg[:], _AF.Sqrt)
            dinv = const.tile([P, NT], FP)
            nc.vector.reciprocal(dinv[:], sdeg[:])
            # sdeg transposed: row t holds sqrt(deg) of tile t's targets
            sdT_ps = p1ps.tile([NT, P], FP, space="PSUM")
            nc.tensor.transpose(sdT_ps[:], sdeg[:], ident[:])
            sdegT = const.tile([NT, P], FP)
            nc.vector.tensor_copy(sdegT[:], sdT_ps[:])
            # matmul lhsT needs base partition 0: flatten the [NT, P] rows
            # into one [1, NT*P] row on partition 0, via a DRAM bounce
            sd_dram = dram.tile([NT, P], FP)
            nc.sync.dma_start(sd_dram[:], sdegT[:])
            sdegrows = const.tile([1, NT * P], FP)
            nc.sync.dma_start(sdegrows[:],
                              sd_dram[:].rearrange("(a t) p -> a (t p)", a=1))

        # decoder bias broadcast [128,1]
        with tc.tile_pool(name="p1b", bufs=1, space="PSUM") as p1b:
            db_ps = p1b.tile([P, 1], FP, space="PSUM")
            nc.tensor.matmul(db_ps[:], lhsT=ones1[:], rhs=dbsb[:],
                             start=True, stop=True)
            decb = const.tile([P, 1], FP)
            nc.vector.tensor_copy(decb[:], db_ps[:])

        # ---- phase 2: G1 = dinv * (x @ W1) for own nodes ------------
        with tc.tile_pool(name="p2", bufs=3) as p2, \
             tc.tile_pool(name="p2ps", bufs=3, space="PSUM") as p2ps:
            xt = p2.tile([P, 2, NLOC], FP, tag="xt", bufs=1)
            nc.sync.dma_start(xt[:, 0, :], xT_in[0:P, :])
            nc.sync.dma_start(xt[:, 1, :], xT_in[P:IN_DIM, :])
            for c in range(NT):
                hps = p2ps.tile([P, HID], FP, space="PSUM", tag="hps")
                nc.tensor.matmul(hps[:], lhsT=xt[:, 0, bass.ts(c, P)],
                                 rhs=w1sb[:, 0, :], start=True, stop=False)
                nc.tensor.matmul(hps[:], lhsT=xt[:, 1, bass.ts(c, P)],
                                 rhs=w1sb[:, 1, :], start=False, stop=True)
                g1t = p2.tile([P, HID], FP, tag="g1t")
                nc.scalar.activation(g1t[:], hps[:], _AF.Copy,
                                     scale=dinv[:, c:c + 1])
                nc.sync.dma_start(g1loc[bass.ts(c, P), :], g1t[:])
        nc.gpsimd.collective_compute(
            "AllGather", _OP.bypass, replica_groups=[list(range(NCORES))],
            ins=[g1loc[:].opt()], outs=[g1full[:].opt()])

        # ---- phases 3 & 4: the two aggregation layers ---------------
        def aggregate(gfull, layer):
            """One-hot matmul aggregation over all 16 target tiles.
            layer==1: emit G2 tiles -> g2loc. layer==2: emit zT -> ztloc."""
            with tc.tile_pool(name=f"ag{layer}", bufs=2) as ag, \
                 tc.tile_pool(name=f"ag{layer}ps", bufs=4, space="PSUM") as agps, \
                 tc.tile_pool(name=f"ag{layer}o", bufs=3) as ago:
                if layer == 2:
                    ztsb = ago.tile([EMB, NLOC], FP, tag="ztsb", bufs=1)
                for k in range(NGATHER):
                    gath = ag.tile([P, GB, HID], FP, tag="gath")
                    nc.gpsimd.dma_gather(
                        gath[:, :, :], gfull[:],
                        idxs[:, k * (NIG // 16):(k + 1) * (NIG // 16)],
                        NIG, NIG, HID, elem_step=HID,
                        # >1008 idxs overflows the 64-desc/engine packet
                        single_packet=False)
                    for lt in range(TILES_PER_GATHER):
                        t = k * TILES_PER_GATHER + lt
                        aps = agps.tile([P, HID], FP, space="PSUM", tag="aps",
                                        bufs=2)
                        for b in range(NBT):
                            gb = t * NBT + b
                            oh = ag.tile([P, P], FP, tag="oh", bufs=4)
                            nc.vector.tensor_scalar(
                                out=oh[:], in0=iota[:],
                                scalar1=colv[:, gb:gb + 1],
                                scalar2=wv[:, gb:gb + 1],
                                op0=_OP.is_equal, op1=_OP.mult)
                            nc.tensor.matmul(
                                aps[:], lhsT=oh[:], rhs=gath[:, lt * NBT + b, :],
                                start=(b == 0),
                                stop=(layer == 2 and b == NBT - 1))
                        if layer == 1:
                            # += sqrt(deg)[t] (x) b1 so that
                            # relu(dinv*psum + b1) comes out of one ACT op
                            nc.tensor.matmul(
                                aps[:], lhsT=sdegrows[:, bass.ts(t, P)],
                                rhs=b1sb[:], start=False, stop=True)
                            h2 = ago.tile([P, HID], FP, tag="h2")
                            nc.scalar.activation(h2[:], aps[:], _AF.Relu,
                                                 scale=dinv[:, t:t + 1])
                            g2t = ago.tile([P, HID], FP, tag="g2t")
                            nc.vector.tensor_scalar_mul(g2t[:], h2[:],
                                                        dinv[:, t:t + 1])
                            nc.sync.dma_start(g2loc[bass.ts(t, P), :], g2t[:])
                        else:
                            a2d = ago.tile([P, HID], FP, tag="a2d")
                            nc.scalar.activation(a2d[:], aps[:], _AF.Copy,
                                                 scale=dinv[:, t:t + 1])
                            tps = agps.tile([P, P], FP, space="PSUM", tag="tps",
                                            bufs=2)
                            nc.tensor.transpose(tps[:], a2d[:], ident[:])
                            a2dT = ago.tile([P, P], FP, tag="a2dT")
                            nc.vector.tensor_copy(a2dT[:], tps[:])
                            zps = agps.tile([EMB, P], FP, space="PSUM", tag="zps",
                                            bufs=2)
                            nc.tensor.matmul(zps[:], lhsT=w2sb[:],
                                             rhs=a2dT[:], start=True, stop=True)
                            nc.scalar.activation(ztsb[:, bass.ts(t, P)], zps[:],
                                                 _AF.Identity, bias=b2sb[:, :1])
                if layer == 2:
                    nc.sync.dma_start(ztloc[:], ztsb[:])
                    nc.sync.dma_start(z_out[:], ztsb[:])

        aggregate(g1full, 1)
        nc.gpsimd.collective_compute(
            "AllGather", _OP.bypass, replica_groups=[list(range(NCORES))],
            ins=[g2loc[:].opt()], outs=[g2full[:].opt()])

        aggregate(g2full, 2)
        nc.gpsimd.collective_compute(
            "AllGather", _OP.bypass, replica_groups=[list(range(NCORES))],
            ins=[ztloc[:].opt()], outs=[ztstack[:].opt()])

        # ---- phase 5: decoder od = softplus(z @ z.T + db) -----------
        OCH = 8192            # output buffer width
        DCH = 2048            # psum / ACT chunk width
        with tc.tile_pool(name="p5", bufs=2) as p5, \
             tc.tile_pool(name="p5ps", bufs=2, space="PSUM") as p5ps:
            ztfull = p5.tile([EMB, N], FP, tag="ztfull", bufs=1)
            for d in range(NCORES):
                nc.sync.dma_start(ztfull[:, bass.ts(d, NLOC)],
                                  ztstack[bass.ts(d, EMB), :])
            # this core's own zT rows for the stationary side
            ztmine = p5.tile([EMB, NLOC], FP, tag="ztmine", bufs=1)
            nc.sync.dma_start(ztmine[:], ztloc[:])
            for m in range(NT):
                for h in range(N // OCH):
                    obuf = p5.tile([P, OCH], FP, tag="obuf")
                    for q in range(OCH // DCH):
                        dps = p5ps.tile([P, DCH], FP, space="PSUM", tag="dps")
                        for s in range(DCH // 512):
                            n0 = h * OCH + q * DCH + s * 512
                            nc.tensor.matmul(
                                dps[:, bass.ts(s, 512)],
                                lhsT=ztmine[:, bass.ts(m, P)],
                                rhs=ztfull[:, n0:n0 + 512],
                                start=True, stop=True)
                        tbuf = p5.tile([P, DCH], FP, tag="tbuf", bufs=3)
                        nc.scalar.activation(tbuf[:], dps[:], _AF.Exp,
                                             bias=decb[:, :1])
                        nc.scalar.activation(obuf[:, bass.ts(q, DCH)], tbuf[:],
                                             _AF.Ln, bias=1.0)
                    nc.sync.dma_start(
                        od_out[bass.ts(m, P), bass.ts(h, OCH)], obuf[:])

    nc.compile()
    return nc


_BUILD_CACHE = {}


def kernel(x, edge_index, edge_weight, W1, b1, W2, b2, dec_bias):
    x = np.asarray(x, np.float32)
    edge_index = np.asarray(edge_index, np.int64)
    edge_weight = np.asarray(edge_weight, np.float32)
    W1 = np.asarray(W1, np.float32)
    b1 = np.asarray(b1, np.float32)
    W2 = np.asarray(W2, np.float32)
    b2 = np.asarray(b2, np.float32)
    dec_bias = np.asarray(dec_bias, np.float32)

    per_core, NBT, DMAX = _prep_inputs(x, edge_index[0], edge_index[1],
                                       edge_weight)

    key = (NBT, DMAX)
    if key not in _BUILD_CACHE:
        _BUILD_CACHE[key] = _build(NBT, DMAX)
    nc = _BUILD_CACHE[key]

    in_maps = []
    for c in range(NCORES):
        pc = per_core[c]
        in_maps.append({
            "xT_in": np.ascontiguousarray(
                x[c * NLOC:(c + 1) * NLOC, :].T),
            "w1_in": W1,
            "b1_in": b1.reshape(1, HID),
            "w2_in": W2,
            "b2_in": b2.reshape(EMB, 1),
            "db_in": dec_bias.reshape(1, 1),
            "idx_in": pc["idx"],
            "col_in": pc["colv"],
            "wv_in": pc["wv"],
            "wcsr_in": pc["wcsr"],
        })

    trace = os.environ.get("GCN_TRACE") == "1"
    r = run_bass_kernel_spmd(nc, in_maps, core_ids=list(range(NCORES)),
                             trace=trace)
    if trace and r.exec_time_ns is not None:
        print(f"HW exec time: {r.exec_time_ns} ns")

    od = np.concatenate([r.results[c]["od_out"] for c in range(NCORES)],
                        axis=0)
    z = np.concatenate([r.results[c]["z_out"].T for c in range(NCORES)],
                       axis=0)
    return od, z


# revision 12
# speedup vs baseline: 1.3162x; 1.3162x over previous
"""GCN autoencoder (2-layer GCNConv encoder + inner-product decoder) on
8 Trainium2 NeuronCores.

Strategy (graph/data parallel, per the node-range sharding):
  - Nodes are sharded 2048/core by TARGET (col) range; edges are bucketed
    host-side by target tile (128 targets) and padded to a static block
    count so all 8 cores run one shared NEFF.
  - deg/dinv are computed on-device from a host-laid-out padded-CSR of
    edge weights (placement is host index work; all value math on device).
  - GCNConv is decomposed as out[c] = dinv[c] * sum_e w_e * G[row_e] + b
    with G[r] = dinv[r] * (x W)[r], so no per-edge dinv gathers are needed.
  - The scatter-add is a dense one-hot matmul: for each block of 128 edges
    (all targeting one 128-target tile), build onehot[e,t] = w_e*(col_e==t)
    with iota + tensor_scalar(is_equal, mult), gather the 128 source rows
    of G with one bulk dma_gather, and accumulate PSUM[t,f] += onehot.T @ G.
  - Full G is assembled between layers with an AllGather collective.
  - Decoder: zT ([64,16384]) lives in SBUF; od rows are sharded per core;
    softplus is Ln(1+Exp(x)) (both funcs in one ACT table).
"""

import os
import numpy as np
import ml_dtypes

import concourse.bass as bass
import concourse.tile as tile
from concourse import bacc, mybir
from concourse.bass_utils import run_bass_kernel_spmd
from concourse.masks import make_identity
from contextlib import ExitStack

P = 128
NCORES = 8
N = 16384
IN_DIM = 256
HID = 128
EMB = 64
E = 524288
NLOC = N // NCORES            # 2048 nodes per core
NT = NLOC // P                # 16 target tiles per core
TILES_PER_GATHER = 2          # gather granularity (tiles)

FP = mybir.dt.float32
BF = mybir.dt.bfloat16
I16 = mybir.dt.int16

_AF = mybir.ActivationFunctionType
_OP = mybir.AluOpType


# ----------------------------------------------------------------------
# host-side layout prep (index work only; all value math runs on device)
# ----------------------------------------------------------------------

def _prep_inputs(x, row, col, w):
    """Bucket edges by target tile, pad to a static block count, and build
    the per-core input arrays."""
    rows_all = np.concatenate([row, np.arange(N, dtype=np.int64)])
    cols_all = np.concatenate([col, np.arange(N, dtype=np.int64)])
    w_all = np.concatenate([w, np.ones(N, np.float32)]).astype(np.float32)

    tile_id = (cols_all // P).astype(np.int64)          # 0..127 global tiles
    order = np.argsort(tile_id, kind="stable")
    rows_s, cols_s, w_s = rows_all[order], cols_all[order], w_all[order]
    tile_s = tile_id[order]
    counts = np.bincount(tile_s, minlength=N // P)       # edges per tile
    NBT = int(np.ceil(counts.max() / P))                 # blocks per tile
    starts = np.concatenate([[0], np.cumsum(counts)])

    # per-target degree CSR width
    deg_counts = np.bincount(cols_all, minlength=N)
    DMAX = int(deg_counts.max())

    per_core = []
    NB_TOT = NT * NBT
    NI = NB_TOT * P
    for c in range(NCORES):
        idx_flat = np.zeros(NI, np.int64)
        colv = np.zeros((P, NB_TOT), np.float32)
        wv = np.zeros((P, NB_TOT), np.float32)
        wcsr = np.zeros((P, NT, DMAX), np.float32)
        for lt in range(NT):
            t = c * NT + lt
            s, e = starts[t], starts[t + 1]
            cnt = e - s
            base = lt * NBT * P
            idx_flat[base:base + cnt] = rows_s[s:e]
            cl = (cols_s[s:e] - t * P).astype(np.float32)
            ww = w_s[s:e]
            flat_cl = np.zeros(NBT * P, np.float32)
            flat_w = np.zeros(NBT * P, np.float32)
            flat_cl[:cnt] = cl
            flat_w[:cnt] = ww
            colv[:, lt * NBT:(lt + 1) * NBT] = flat_cl.reshape(NBT, P).T
            wv[:, lt * NBT:(lt + 1) * NBT] = flat_w.reshape(NBT, P).T
            # padded-CSR of weights per target for the degree reduce
            tgt_local = (cols_s[s:e] - t * P).astype(np.int64)
            slot = np.zeros(P, np.int64)
            for j in range(cnt):
                p = tgt_local[j]
                wcsr[p, lt, slot[p]] = ww[j]
                slot[p] += 1
        # dma_gather idx layout: unwrapped[i] = idxs[i%16, i//16],
        # replicated across the 8 groups of 16 partitions
        idx16 = np.zeros((16, NI // 16), np.int16)
        idx16[np.arange(NI) % 16, np.arange(NI) // 16] = idx_flat.astype(np.int16)
        idx_lay = np.tile(idx16, (8, 1))
        per_core.append(dict(idx=idx_lay,
                             colv=colv, wv=wv,
                             wcsr=wcsr.reshape(P, NT * DMAX)))
    return per_core, NBT, DMAX


# ----------------------------------------------------------------------
# device kernel
# ----------------------------------------------------------------------

def _build(NBT, DMAX):
    NB_TOT = NT * NBT
    NI = NB_TOT * P
    GB = TILES_PER_GATHER * NBT          # blocks per gather
    NIG = GB * P                         # idxs per gather
    NGATHER = NT // TILES_PER_GATHER

    nc = bacc.Bacc("TRN2", target_bir_lowering=False, debug=False,
                   num_devices=NCORES, num_swdge_queues=4)

    xT_in = nc.dram_tensor("xT_in", [IN_DIM, NLOC], FP, kind="ExternalInput").ap()
    w1_in = nc.dram_tensor("w1_in", [IN_DIM, HID], FP, kind="ExternalInput").ap()
    b1_in = nc.dram_tensor("b1_in", [1, HID], FP, kind="ExternalInput").ap()
    w2_in = nc.dram_tensor("w2_in", [HID, EMB], FP, kind="ExternalInput").ap()
    b2_in = nc.dram_tensor("b2_in", [EMB, 1], FP, kind="ExternalInput").ap()
    db_in = nc.dram_tensor("db_in", [1, 1], FP, kind="ExternalInput").ap()
    idx_in = nc.dram_tensor("idx_in", [P, NI // 16], I16, kind="ExternalInput").ap()
    col_in = nc.dram_tensor("col_in", [P, NB_TOT], FP, kind="ExternalInput").ap()
    wv_in = nc.dram_tensor("wv_in", [P, NB_TOT], FP, kind="ExternalInput").ap()
    wcsr_in = nc.dram_tensor("wcsr_in", [P, NT * DMAX], FP, kind="ExternalInput").ap()

    od_out = nc.dram_tensor("od_out", [NLOC, N], FP, kind="ExternalOutput").ap()
    z_out = nc.dram_tensor("z_out", [EMB, NLOC], FP, kind="ExternalOutput").ap()

    with tile.TileContext(nc) as tc, ExitStack() as ctx:
        dram = ctx.enter_context(tc.tile_pool(name="dram", bufs=1, space="DRAM"))
        g1loc = dram.tile([NLOC, HID], BF)
        g1full = dram.tile([N, HID], BF, addr_space="Shared")
        g2loc = dram.tile([NLOC, HID], BF)
        g2full = dram.tile([N, HID], BF, addr_space="Shared")
        ztloc = dram.tile([EMB, NLOC], FP)
        ztstack = dram.tile([NCORES * EMB, NLOC], FP, addr_space="Shared")

        # ---- persistent small tiles ---------------------------------
        const = ctx.enter_context(tc.tile_pool(name="const", bufs=1))
        iota = const.tile([P, P], FP)
        nc.gpsimd.iota(iota[:], pattern=[[1, P]], base=0, channel_multiplier=0,
                       allow_small_or_imprecise_dtypes=True)
        ident = const.tile([P, P], FP)
        make_identity(nc, ident[:])
        iotab = const.tile([P, P], BF)
        nc.vector.tensor_copy(iotab[:], iota[:])
        ones1 = const.tile([1, P], FP)
        nc.vector.memset(ones1[:], 1.0)

        w1sb = const.tile([P, 2, HID], FP)   # [k-half][kp, f1]
        nc.sync.dma_start(w1sb[:, 0, :], w1_in[0:P, :])
        nc.sync.dma_start(w1sb[:, 1, :], w1_in[P:IN_DIM, :])
        w2sb = const.tile([HID, EMB], FP)
        nc.sync.dma_start(w2sb[:], w2_in[:])
        b1sb = const.tile([1, HID], FP)
        nc.sync.dma_start(b1sb[:], b1_in[:])
        b2sb = const.tile([EMB, 1], FP)
        nc.sync.dma_start(b2sb[:], b2_in[:])
        dbsb = const.tile([1, 1], FP)
        nc.sync.dma_start(dbsb[:], db_in[:])

        colv = const.tile([P, NB_TOT], FP)
        nc.sync.dma_start(colv[:], col_in[:])
        wv = const.tile([P, NB_TOT], FP)
        nc.sync.dma_start(wv[:], wv_in[:])
        idxs = const.tile([P, NI // 16], I16)
        nc.sync.dma_start(idxs[:], idx_in[:])

        # ---- phase 1: degree -> dinv --------------------------------
        with tc.tile_pool(name="p1", bufs=1) as p1, \
             tc.tile_pool(name="p1ps", bufs=1, space="PSUM") as p1ps:
            wcsr = p1.tile([P, NT, DMAX], FP)
            nc.sync.dma_start(wcsr[:, :, :], wcsr_in[:].rearrange(
                "p (t d) -> p t d", t=NT))
            deg = const.tile([P, NT], FP)
            nc.vector.tensor_reduce(deg[:], wcsr[:, :, :],
                                    axis=mybir.AxisListType.X, op=_OP.add)
            sdeg = const.tile([P, NT], FP)
            nc.scalar.activation(sdeg[:], deg[:], _AF.Sqrt)
            dinv = const.tile([P, NT], FP)
            nc.vector.reciprocal(dinv[:], sdeg[:])
            # sdeg transposed: row t holds sqrt(deg) of tile t's targets
            sdT_ps = p1ps.tile([NT, P], FP, space="PSUM")
            nc.tensor.transpose(sdT_ps[:], sdeg[:], ident[:])
            sdegT = const.tile([NT, P], FP)
            nc.vector.tensor_copy(sdegT[:], sdT_ps[:])
            # matmul lhsT needs base partition 0: flatten the [NT, P] rows
            # into one [1, NT*P] row on partition 0, via a DRAM bounce
            sd_dram = dram.tile([NT, P], FP)
            nc.sync.dma_start(sd_dram[:], sdegT[:])
            sdegrows = const.tile([1, NT * P], FP)
            nc.sync.dma_start(sdegrows[:],
                              sd_dram[:].rearrange("(a t) p -> a (t p)", a=1))

        # decoder bias broadcast [128,1]
        with tc.tile_pool(name="p1b", bufs=1, space="PSUM") as p1b:
            db_ps = p1b.tile([P, 1], FP, space="PSUM")
            nc.tensor.matmul(db_ps[:], lhsT=ones1[:], rhs=dbsb[:],
                             start=True, stop=True)
            decb = const.tile([P, 1], FP)
            nc.vector.tensor_copy(decb[:], db_ps[:])

        # ---- phase 2: G1 = dinv * (x @ W1) for own nodes ------------
        with tc.tile_pool(name="p2", bufs=3) as p2, \
             tc.tile_pool(name="p2ps", bufs=3, space="PSUM") as p2ps:
            xt = p2.tile([P, 2, NLOC], FP, tag="xt", bufs=1)
            nc.sync.dma_start(xt[:, 0, :], xT_in[0:P, :])
            nc.sync.dma_start(xt[:, 1, :], xT_in[P:IN_DIM, :])
            for c in range(NT):
                hps = p2ps.tile([P, HID], FP, space="PSUM", tag="hps")
                nc.tensor.matmul(hps[:], lhsT=xt[:, 0, bass.ts(c, P)],
                                 rhs=w1sb[:, 0, :], start=True, stop=False)
                nc.tensor.matmul(hps[:], lhsT=xt[:, 1, bass.ts(c, P)],
                                 rhs=w1sb[:, 1, :], start=False, stop=True)
                g1t = p2.tile([P, HID], BF, tag="g1t")
                nc.scalar.activation(g1t[:], hps[:], _AF.Copy,
                                     scale=dinv[:, c:c + 1])
                nc.sync.dma_start(g1loc[bass.ts(c, P), :], g1t[:])
        nc.gpsimd.collective_compute(
            "AllGather", _OP.bypass, replica_groups=[list(range(NCORES))],
            ins=[g1loc[:].opt()], outs=[g1full[:].opt()])

        # ---- phases 3 & 4: the two aggregation layers ---------------
        def aggregate(gfull, layer):
            """One-hot matmul aggregation over all 16 target tiles.
            layer==1: emit G2 tiles -> g2loc. layer==2: emit zT -> ztloc."""
            with tc.tile_pool(name=f"ag{layer}", bufs=2) as ag, \
                 tc.tile_pool(name=f"ag{layer}ps", bufs=4, space="PSUM") as agps, \
                 tc.tile_pool(name=f"ag{layer}o", bufs=3) as ago:
                if layer == 2:
                    ztsb = ago.tile([EMB, NLOC], FP, tag="ztsb", bufs=1)
                for k in range(NGATHER):
                    gath = ag.tile([P, GB, HID], BF, tag="gath")
                    nc.gpsimd.dma_gather(
                        gath[:, :, :], gfull[:],
                        idxs[:, k * (NIG // 16):(k + 1) * (NIG // 16)],
                        NIG, NIG, HID, elem_step=HID,
                        # >1008 idxs overflows the 64-desc/engine packet
                        single_packet=False, queue_num=k % 4)
                    for lt in range(TILES_PER_GATHER):
                        t = k * TILES_PER_GATHER + lt
                        aps = agps.tile([P, HID], FP, space="PSUM", tag="aps",
                                        bufs=2)
                        for b in range(NBT):
                            gb = t * NBT + b
                            oh = ag.tile([P, P], BF, tag="oh", bufs=4)
                            nc.vector.tensor_scalar(
                                out=oh[:], in0=iotab[:],
                                scalar1=colv[:, gb:gb + 1],
                                scalar2=wv[:, gb:gb + 1],
                                op0=_OP.is_equal, op1=_OP.mult)
                            nc.tensor.matmul(
                                aps[:], lhsT=oh[:], rhs=gath[:, lt * NBT + b, :],
                                start=(b == 0),
                                stop=(layer == 2 and b == NBT - 1))
                        if layer == 1:
                            # += sqrt(deg)[t] (x) b1 so that
                            # relu(dinv*psum + b1) comes out of one ACT op
                            nc.tensor.matmul(
                                aps[:], lhsT=sdegrows[:, bass.ts(t, P)],
                                rhs=b1sb[:], start=False, stop=True)
                            h2 = ago.tile([P, HID], FP, tag="h2")
                            nc.scalar.activation(h2[:], aps[:], _AF.Relu,
                                                 scale=dinv[:, t:t + 1])
                            g2t = ago.tile([P, HID], BF, tag="g2t")
                            nc.vector.tensor_scalar_mul(g2t[:], h2[:],
                                                        dinv[:, t:t + 1])
                            nc.sync.dma_start(g2loc[bass.ts(t, P), :], g2t[:])
                        else:
                            a2d = ago.tile([P, HID], FP, tag="a2d")
                            nc.scalar.activation(a2d[:], aps[:], _AF.Copy,
                                                 scale=dinv[:, t:t + 1])
                            tps = agps.tile([P, P], FP, space="PSUM", tag="tps",
                                            bufs=2)
                            nc.tensor.transpose(tps[:], a2d[:], ident[:])
                            a2dT = ago.tile([P, P], FP, tag="a2dT")
                            nc.vector.tensor_copy(a2dT[:], tps[:])
                            zps = agps.tile([EMB, P], FP, space="PSUM", tag="zps",
                                            bufs=2)
                            nc.tensor.matmul(zps[:], lhsT=w2sb[:],
                                             rhs=a2dT[:], start=True, stop=True)
                            nc.scalar.activation(ztsb[:, bass.ts(t, P)], zps[:],
                                                 _AF.Identity, bias=b2sb[:, :1])
                if layer == 2:
                    nc.sync.dma_start(ztloc[:], ztsb[:])
                    nc.sync.dma_start(z_out[:], ztsb[:])

        aggregate(g1full, 1)
        nc.gpsimd.collective_compute(
            "AllGather", _OP.bypass, replica_groups=[list(range(NCORES))],
            ins=[g2loc[:].opt()], outs=[g2full[:].opt()])

        aggregate(g2full, 2)
        nc.gpsimd.collective_compute(
            "AllGather", _OP.bypass, replica_groups=[list(range(NCORES))],
            ins=[ztloc[:].opt()], outs=[ztstack[:].opt()])

        # ---- phase 5: decoder od = softplus(z @ z.T + db) -----------
        OCH = 8192            # output buffer width
        DCH = 2048            # psum / ACT chunk width
        with tc.tile_pool(name="p5", bufs=2) as p5, \
             tc.tile_pool(name="p5ps", bufs=2, space="PSUM") as p5ps:
            ztfull = p5.tile([EMB, N], BF, tag="ztfull", bufs=1)
            for d in range(NCORES):
                nc.gpsimd.dma_start(ztfull[:, bass.ts(d, NLOC)],
                                    ztstack[bass.ts(d, EMB), :])
            # this core's own zT rows for the stationary side
            ztmine = p5.tile([EMB, NLOC], BF, tag="ztmine", bufs=1)
            nc.gpsimd.dma_start(ztmine[:], ztloc[:])
            for m in range(NT):
                for h in range(N // OCH):
                    obuf = p5.tile([P, OCH], FP, tag="obuf")
                    tbuf = p5.tile([P, OCH], FP, tag="tbuf", bufs=1)
                    for q in range(OCH // DCH):
                        dps = p5ps.tile([P, DCH], FP, space="PSUM", tag="dps")
                        for s in range(DCH // 512):
                            n0 = h * OCH + q * DCH + s * 512
                            nc.tensor.matmul(
                                dps[:, bass.ts(s, 512)],
                                lhsT=ztmine[:, bass.ts(m, P)],
                                rhs=ztfull[:, n0:n0 + 512],
                                start=True, stop=True)
                        # batch all Exp ops, then one wide Ln: avoids
                        # per-op ACT table thrash (Exp and Ln live in
                        # different default table sets)
                        nc.scalar.activation(tbuf[:, bass.ts(q, DCH)], dps[:],
                                             _AF.Exp, bias=decb[:, :1])
                    nc.scalar.activation(obuf[:], tbuf[:], _AF.Ln, bias=1.0)
                    nc.sync.dma_start(
                        od_out[bass.ts(m, P), bass.ts(h, OCH)], obuf[:])

    nc.compile()
    return nc


_BUILD_CACHE = {}


def kernel(x, edge_index, edge_weight, W1, b1, W2, b2, dec_bias):
    x = np.asarray(x, np.float32)
    edge_index = np.asarray(edge_index, np.int64)
    edge_weight = np.asarray(edge_weight, np.float32)
    W1 = np.asarray(W1, np.float32)
    b1 = np.asarray(b1, np.float32)
    W2 = np.asarray(W2, np.float32)
    b2 = np.asarray(b2, np.float32)
    dec_bias = np.asarray(dec_bias, np.float32)

    per_core, NBT, DMAX = _prep_inputs(x, edge_index[0], edge_index[1],
                                       edge_weight)

    key = (NBT, DMAX)
    if key not in _BUILD_CACHE:
        _BUILD_CACHE[key] = _build(NBT, DMAX)
    nc = _BUILD_CACHE[key]

    in_maps = []
    for c in range(NCORES):
        pc = per_core[c]
        in_maps.append({
            "xT_in": np.ascontiguousarray(
                x[c * NLOC:(c + 1) * NLOC, :].T),
            "w1_in": W1,
            "b1_in": b1.reshape(1, HID),
            "w2_in": W2,
            "b2_in": b2.reshape(EMB, 1),
            "db_in": dec_bias.reshape(1, 1),
            "idx_in": pc["idx"],
            "col_in": pc["colv"],
            "wv_in": pc["wv"],
            "wcsr_in": pc["wcsr"],
        })

    trace = os.environ.get("GCN_TRACE") == "1"
    r = run_bass_kernel_spmd(nc, in_maps, core_ids=list(range(NCORES)),
                             trace=trace)
    if trace and r.exec_time_ns is not None:
        print(f"HW exec time: {r.exec_time_ns} ns")

    od = np.concatenate([r.results[c]["od_out"] for c in range(NCORES)],
                        axis=0)
    z = np.concatenate([r.results[c]["z_out"].T for c in range(NCORES)],
                       axis=0)
    return od, z


# revision 13
# speedup vs baseline: 1.4103x; 1.0715x over previous
"""GCN autoencoder (2-layer GCNConv encoder + inner-product decoder) on
8 Trainium2 NeuronCores.

Strategy (graph/data parallel, per the node-range sharding):
  - Nodes are sharded 2048/core by TARGET (col) range; edges are bucketed
    host-side by target tile (128 targets) and padded to a static block
    count so all 8 cores run one shared NEFF.
  - deg/dinv are computed on-device from a host-laid-out padded-CSR of
    edge weights (placement is host index work; all value math on device).
  - GCNConv is decomposed as out[c] = dinv[c] * sum_e w_e * G[row_e] + b
    with G[r] = dinv[r] * (x W)[r], so no per-edge dinv gathers are needed.
  - The scatter-add is a dense one-hot matmul: for each block of 128 edges
    (all targeting one 128-target tile), build onehot[e,t] = w_e*(col_e==t)
    with iota + tensor_scalar(is_equal, mult), gather the 128 source rows
    of G with one bulk dma_gather, and accumulate PSUM[t,f] += onehot.T @ G.
  - Full G is assembled between layers with an AllGather collective.
  - Decoder: zT ([64,16384]) lives in SBUF; od rows are sharded per core;
    softplus is Ln(1+Exp(x)) (both funcs in one ACT table).
"""

import os
import numpy as np
import ml_dtypes

import concourse.bass as bass
import concourse.tile as tile
from concourse import bacc, mybir
from concourse.bass_utils import run_bass_kernel_spmd
from concourse.masks import make_identity
from contextlib import ExitStack

P = 128
NCORES = 8
N = 16384
IN_DIM = 256
HID = 128
EMB = 64
E = 524288
NLOC = N // NCORES            # 2048 nodes per core
NT = NLOC // P                # 16 target tiles per core
TILES_PER_GATHER = 2          # gather granularity (tiles)

FP = mybir.dt.float32
BF = mybir.dt.bfloat16
I16 = mybir.dt.int16

_AF = mybir.ActivationFunctionType
_OP = mybir.AluOpType


# ----------------------------------------------------------------------
# host-side layout prep (index work only; all value math runs on device)
# ----------------------------------------------------------------------

def _prep_inputs(x, row, col, w):
    """Bucket edges by target tile, pad to a static block count, and build
    the per-core input arrays."""
    rows_all = np.concatenate([row, np.arange(N, dtype=np.int64)])
    cols_all = np.concatenate([col, np.arange(N, dtype=np.int64)])
    w_all = np.concatenate([w, np.ones(N, np.float32)]).astype(np.float32)

    tile_id = (cols_all // P).astype(np.int64)          # 0..127 global tiles
    order = np.argsort(tile_id, kind="stable")
    rows_s, cols_s, w_s = rows_all[order], cols_all[order], w_all[order]
    tile_s = tile_id[order]
    counts = np.bincount(tile_s, minlength=N // P)       # edges per tile
    NBT = int(np.ceil(counts.max() / P))                 # blocks per tile
    starts = np.concatenate([[0], np.cumsum(counts)])

    # per-target degree CSR width
    deg_counts = np.bincount(cols_all, minlength=N)
    DMAX = int(deg_counts.max())

    per_core = []
    NB_TOT = NT * NBT
    NI = NB_TOT * P
    for c in range(NCORES):
        idx_flat = np.zeros(NI, np.int64)
        colv = np.zeros((P, NB_TOT), np.float32)
        wv = np.zeros((P, NB_TOT), np.float32)
        wcsr = np.zeros((P, NT, DMAX), np.float32)
        for lt in range(NT):
            t = c * NT + lt
            s, e = starts[t], starts[t + 1]
            cnt = e - s
            base = lt * NBT * P
            idx_flat[base:base + cnt] = rows_s[s:e]
            cl = (cols_s[s:e] - t * P).astype(np.float32)
            ww = w_s[s:e]
            flat_cl = np.zeros(NBT * P, np.float32)
            flat_w = np.zeros(NBT * P, np.float32)
            flat_cl[:cnt] = cl
            flat_w[:cnt] = ww
            colv[:, lt * NBT:(lt + 1) * NBT] = flat_cl.reshape(NBT, P).T
            wv[:, lt * NBT:(lt + 1) * NBT] = flat_w.reshape(NBT, P).T
            # padded-CSR of weights per target for the degree reduce
            tgt_local = (cols_s[s:e] - t * P).astype(np.int64)
            slot = np.zeros(P, np.int64)
            for j in range(cnt):
                p = tgt_local[j]
                wcsr[p, lt, slot[p]] = ww[j]
                slot[p] += 1
        # dma_gather idx layout: unwrapped[i] = idxs[i%16, i//16],
        # replicated across the 8 groups of 16 partitions
        idx16 = np.zeros((16, NI // 16), np.int16)
        idx16[np.arange(NI) % 16, np.arange(NI) // 16] = idx_flat.astype(np.int16)
        idx_lay = np.tile(idx16, (8, 1))
        per_core.append(dict(idx=idx_lay,
                             colv=colv, wv=wv,
                             wcsr=wcsr.reshape(P, NT * DMAX)))
    return per_core, NBT, DMAX


# ----------------------------------------------------------------------
# device kernel
# ----------------------------------------------------------------------

def _build(NBT, DMAX):
    NB_TOT = NT * NBT
    NI = NB_TOT * P
    GB = TILES_PER_GATHER * NBT          # blocks per gather
    NIG = GB * P                         # idxs per gather
    NGATHER = NT // TILES_PER_GATHER

    nc = bacc.Bacc("TRN2", target_bir_lowering=False, debug=False,
                   num_devices=NCORES, num_swdge_queues=4)

    xT_in = nc.dram_tensor("xT_in", [IN_DIM, NLOC], FP, kind="ExternalInput").ap()
    w1_in = nc.dram_tensor("w1_in", [IN_DIM, HID], FP, kind="ExternalInput").ap()
    b1_in = nc.dram_tensor("b1_in", [1, HID], FP, kind="ExternalInput").ap()
    w2_in = nc.dram_tensor("w2_in", [HID, EMB], FP, kind="ExternalInput").ap()
    b2_in = nc.dram_tensor("b2_in", [EMB, 1], FP, kind="ExternalInput").ap()
    db_in = nc.dram_tensor("db_in", [1, 1], FP, kind="ExternalInput").ap()
    idx_in = nc.dram_tensor("idx_in", [P, NI // 16], I16, kind="ExternalInput").ap()
    col_in = nc.dram_tensor("col_in", [P, NB_TOT], FP, kind="ExternalInput").ap()
    wv_in = nc.dram_tensor("wv_in", [P, NB_TOT], FP, kind="ExternalInput").ap()
    wcsr_in = nc.dram_tensor("wcsr_in", [P, NT * DMAX], FP, kind="ExternalInput").ap()

    od_out = nc.dram_tensor("od_out", [NLOC, N], FP, kind="ExternalOutput").ap()
    z_out = nc.dram_tensor("z_out", [EMB, NLOC], FP, kind="ExternalOutput").ap()

    with tile.TileContext(nc) as tc, ExitStack() as ctx:
        dram = ctx.enter_context(tc.tile_pool(name="dram", bufs=1, space="DRAM"))
        g1loc = dram.tile([NLOC, HID], BF)
        g1full = dram.tile([N, HID], BF, addr_space="Shared")
        g2loc = dram.tile([NLOC, HID], BF)
        g2full = dram.tile([N, HID], BF, addr_space="Shared")
        ztloc = dram.tile([EMB, NLOC], FP)
        ztstack = dram.tile([NCORES * EMB, NLOC], FP, addr_space="Shared")

        # ---- persistent small tiles ---------------------------------
        const = ctx.enter_context(tc.tile_pool(name="const", bufs=1))
        iota = const.tile([P, P], FP)
        nc.gpsimd.iota(iota[:], pattern=[[1, P]], base=0, channel_multiplier=0,
                       allow_small_or_imprecise_dtypes=True)
        ident = const.tile([P, P], FP)
        make_identity(nc, ident[:])
        iotab = const.tile([P, P], BF)
        nc.vector.tensor_copy(iotab[:], iota[:])
        ones1 = const.tile([1, P], FP)
        nc.vector.memset(ones1[:], 1.0)

        w1sb = const.tile([P, 2, HID], FP)   # [k-half][kp, f1]
        nc.sync.dma_start(w1sb[:, 0, :], w1_in[0:P, :])
        nc.sync.dma_start(w1sb[:, 1, :], w1_in[P:IN_DIM, :])
        w2sb = const.tile([HID, EMB], FP)
        nc.sync.dma_start(w2sb[:], w2_in[:])
        b1sb = const.tile([1, HID], FP)
        nc.sync.dma_start(b1sb[:], b1_in[:])
        b2sb = const.tile([EMB, 1], FP)
        nc.sync.dma_start(b2sb[:], b2_in[:])
        dbsb = const.tile([1, 1], FP)
        nc.sync.dma_start(dbsb[:], db_in[:])

        colv = const.tile([P, NB_TOT], FP)
        nc.sync.dma_start(colv[:], col_in[:])
        wv = const.tile([P, NB_TOT], FP)
        nc.sync.dma_start(wv[:], wv_in[:])
        idxs = const.tile([P, NI // 16], I16)
        nc.sync.dma_start(idxs[:], idx_in[:])

        # ---- phase 1: degree -> dinv --------------------------------
        with tc.tile_pool(name="p1", bufs=1) as p1, \
             tc.tile_pool(name="p1ps", bufs=1, space="PSUM") as p1ps:
            wcsr = p1.tile([P, NT, DMAX], FP)
            nc.sync.dma_start(wcsr[:, :, :], wcsr_in[:].rearrange(
                "p (t d) -> p t d", t=NT))
            deg = const.tile([P, NT], FP)
            nc.vector.tensor_reduce(deg[:], wcsr[:, :, :],
                                    axis=mybir.AxisListType.X, op=_OP.add)
            sdeg = const.tile([P, NT], FP)
            nc.scalar.activation(sdeg[:], deg[:], _AF.Sqrt)
            dinv = const.tile([P, NT], FP)
            nc.vector.reciprocal(dinv[:], sdeg[:])
            # sdeg transposed: row t holds sqrt(deg) of tile t's targets
            sdT_ps = p1ps.tile([NT, P], FP, space="PSUM")
            nc.tensor.transpose(sdT_ps[:], sdeg[:], ident[:])
            sdegT = const.tile([NT, P], FP)
            nc.vector.tensor_copy(sdegT[:], sdT_ps[:])
            # matmul lhsT needs base partition 0: flatten the [NT, P] rows
            # into one [1, NT*P] row on partition 0, via a DRAM bounce
            sd_dram = dram.tile([NT, P], FP)
            nc.sync.dma_start(sd_dram[:], sdegT[:])
            sdegrows = const.tile([1, NT * P], FP)
            nc.sync.dma_start(sdegrows[:],
                              sd_dram[:].rearrange("(a t) p -> a (t p)", a=1))

        # decoder bias broadcast [128,1]
        with tc.tile_pool(name="p1b", bufs=1, space="PSUM") as p1b:
            db_ps = p1b.tile([P, 1], FP, space="PSUM")
            nc.tensor.matmul(db_ps[:], lhsT=ones1[:], rhs=dbsb[:],
                             start=True, stop=True)
            decb = const.tile([P, 1], FP)
            nc.vector.tensor_copy(decb[:], db_ps[:])

        # ---- phase 2: G1 = dinv * (x @ W1) for own nodes ------------
        with tc.tile_pool(name="p2", bufs=3) as p2, \
             tc.tile_pool(name="p2ps", bufs=3, space="PSUM") as p2ps:
            xt = p2.tile([P, 2, NLOC], FP, tag="xt", bufs=1)
            nc.sync.dma_start(xt[:, 0, :], xT_in[0:P, :])
            nc.sync.dma_start(xt[:, 1, :], xT_in[P:IN_DIM, :])
            for c in range(NT):
                hps = p2ps.tile([P, HID], FP, space="PSUM", tag="hps")
                nc.tensor.matmul(hps[:], lhsT=xt[:, 0, bass.ts(c, P)],
                                 rhs=w1sb[:, 0, :], start=True, stop=False)
                nc.tensor.matmul(hps[:], lhsT=xt[:, 1, bass.ts(c, P)],
                                 rhs=w1sb[:, 1, :], start=False, stop=True)
                g1t = p2.tile([P, HID], BF, tag="g1t")
                nc.scalar.activation(g1t[:], hps[:], _AF.Copy,
                                     scale=dinv[:, c:c + 1])
                nc.sync.dma_start(g1loc[bass.ts(c, P), :], g1t[:])
        nc.gpsimd.collective_compute(
            "AllGather", _OP.bypass, replica_groups=[list(range(NCORES))],
            ins=[g1loc[:].opt()], outs=[g1full[:].opt()])

        # ---- phases 3 & 4: the two aggregation layers ---------------
        def aggregate(gfull, layer):
            """One-hot matmul aggregation over all 16 target tiles.
            layer==1: emit G2 tiles -> g2loc. layer==2: emit zT -> ztloc."""
            with tc.tile_pool(name=f"ag{layer}", bufs=2) as ag, \
                 tc.tile_pool(name=f"ag{layer}ps", bufs=4, space="PSUM") as agps, \
                 tc.tile_pool(name=f"ag{layer}o", bufs=3) as ago:
                if layer == 2:
                    ztsb = ago.tile([EMB, NLOC], FP, tag="ztsb", bufs=1)
                for k in range(NGATHER):
                    gath = ag.tile([P, GB, HID], BF, tag="gath", bufs=3)
                    nc.gpsimd.dma_gather(
                        gath[:, :, :], gfull[:],
                        idxs[:, k * (NIG // 16):(k + 1) * (NIG // 16)],
                        NIG, NIG, HID, elem_step=HID,
                        # >1008 idxs overflows the 64-desc/engine packet
                        single_packet=False, queue_num=1 + (k % 3))
                    for lt in range(TILES_PER_GATHER):
                        t = k * TILES_PER_GATHER + lt
                        aps = agps.tile([P, HID], FP, space="PSUM", tag="aps",
                                        bufs=2)
                        for b in range(NBT):
                            gb = t * NBT + b
                            oh = ag.tile([P, P], BF, tag="oh", bufs=8)
                            nc.vector.tensor_scalar(
                                out=oh[:], in0=iotab[:],
                                scalar1=colv[:, gb:gb + 1],
                                scalar2=wv[:, gb:gb + 1],
                                op0=_OP.is_equal, op1=_OP.mult)
                            nc.tensor.matmul(
                                aps[:], lhsT=oh[:], rhs=gath[:, lt * NBT + b, :],
                                start=(b == 0),
                                stop=(layer == 2 and b == NBT - 1))
                        if layer == 1:
                            # += sqrt(deg)[t] (x) b1 so that
                            # relu(dinv*psum + b1) comes out of one ACT op
                            nc.tensor.matmul(
                                aps[:], lhsT=sdegrows[:, bass.ts(t, P)],
                                rhs=b1sb[:], start=False, stop=True)
                            h2 = ago.tile([P, HID], FP, tag="h2")
                            nc.scalar.activation(h2[:], aps[:], _AF.Relu,
                                                 scale=dinv[:, t:t + 1])
                            g2t = ago.tile([P, HID], BF, tag="g2t")
                            nc.vector.tensor_scalar_mul(g2t[:], h2[:],
                                                        dinv[:, t:t + 1])
                            nc.sync.dma_start(g2loc[bass.ts(t, P), :], g2t[:])
                        else:
                            a2d = ago.tile([P, HID], FP, tag="a2d")
                            nc.scalar.activation(a2d[:], aps[:], _AF.Copy,
                                                 scale=dinv[:, t:t + 1])
                            tps = agps.tile([P, P], FP, space="PSUM", tag="tps",
                                            bufs=2)
                            nc.tensor.transpose(tps[:], a2d[:], ident[:])
                            a2dT = ago.tile([P, P], FP, tag="a2dT")
                            nc.vector.tensor_copy(a2dT[:], tps[:])
                            zps = agps.tile([EMB, P], FP, space="PSUM", tag="zps",
                                            bufs=2)
                            nc.tensor.matmul(zps[:], lhsT=w2sb[:],
                                             rhs=a2dT[:], start=True, stop=True)
                            nc.scalar.activation(ztsb[:, bass.ts(t, P)], zps[:],
                                                 _AF.Identity, bias=b2sb[:, :1])
                if layer == 2:
                    nc.sync.dma_start(ztloc[:], ztsb[:])
                    nc.sync.dma_start(z_out[:], ztsb[:])

        aggregate(g1full, 1)
        nc.gpsimd.collective_compute(
            "AllGather", _OP.bypass, replica_groups=[list(range(NCORES))],
            ins=[g2loc[:].opt()], outs=[g2full[:].opt()])

        aggregate(g2full, 2)
        nc.gpsimd.collective_compute(
            "AllGather", _OP.bypass, replica_groups=[list(range(NCORES))],
            ins=[ztloc[:].opt()], outs=[ztstack[:].opt()])

        # ---- phase 5: decoder od = softplus(z @ z.T + db) -----------
        OCH = 8192            # output buffer width
        DCH = 2048            # psum / ACT chunk width
        with tc.tile_pool(name="p5", bufs=2) as p5, \
             tc.tile_pool(name="p5ps", bufs=2, space="PSUM") as p5ps:
            ztfull = p5.tile([EMB, N], BF, tag="ztfull", bufs=1)
            for d in range(NCORES):
                nc.gpsimd.dma_start(ztfull[:, bass.ts(d, NLOC)],
                                    ztstack[bass.ts(d, EMB), :])
            # this core's own zT rows for the stationary side
            ztmine = p5.tile([EMB, NLOC], BF, tag="ztmine", bufs=1)
            nc.gpsimd.dma_start(ztmine[:], ztloc[:])
            for m in range(NT):
                for h in range(N // OCH):
                    obuf = p5.tile([P, OCH], FP, tag="obuf")
                    tbuf = p5.tile([P, OCH], FP, tag="tbuf", bufs=1)
                    for q in range(OCH // DCH):
                        dps = p5ps.tile([P, DCH], FP, space="PSUM", tag="dps")
                        for s in range(DCH // 512):
                            n0 = h * OCH + q * DCH + s * 512
                            nc.tensor.matmul(
                                dps[:, bass.ts(s, 512)],
                                lhsT=ztmine[:, bass.ts(m, P)],
                                rhs=ztfull[:, n0:n0 + 512],
                                start=True, stop=True)
                        # batch all Exp ops, then one wide Ln: avoids
                        # per-op ACT table thrash (Exp and Ln live in
                        # different default table sets)
                        nc.scalar.activation(tbuf[:, bass.ts(q, DCH)], dps[:],
                                             _AF.Exp, bias=decb[:, :1])
                    nc.scalar.activation(obuf[:], tbuf[:], _AF.Ln, bias=1.0)
                    nc.sync.dma_start(
                        od_out[bass.ts(m, P), bass.ts(h, OCH)], obuf[:])

    nc.compile()
    return nc


_BUILD_CACHE = {}


def kernel(x, edge_index, edge_weight, W1, b1, W2, b2, dec_bias):
    x = np.asarray(x, np.float32)
    edge_index = np.asarray(edge_index, np.int64)
    edge_weight = np.asarray(edge_weight, np.float32)
    W1 = np.asarray(W1, np.float32)
    b1 = np.asarray(b1, np.float32)
    W2 = np.asarray(W2, np.float32)
    b2 = np.asarray(b2, np.float32)
    dec_bias = np.asarray(dec_bias, np.float32)

    per_core, NBT, DMAX = _prep_inputs(x, edge_index[0], edge_index[1],
                                       edge_weight)

    key = (NBT, DMAX)
    if key not in _BUILD_CACHE:
        _BUILD_CACHE[key] = _build(NBT, DMAX)
    nc = _BUILD_CACHE[key]

    in_maps = []
    for c in range(NCORES):
        pc = per_core[c]
        in_maps.append({
            "xT_in": np.ascontiguousarray(
                x[c * NLOC:(c + 1) * NLOC, :].T),
            "w1_in": W1,
            "b1_in": b1.reshape(1, HID),
            "w2_in": W2,
            "b2_in": b2.reshape(EMB, 1),
            "db_in": dec_bias.reshape(1, 1),
            "idx_in": pc["idx"],
            "col_in": pc["colv"],
            "wv_in": pc["wv"],
            "wcsr_in": pc["wcsr"],
        })

    trace = os.environ.get("GCN_TRACE") == "1"
    r = run_bass_kernel_spmd(nc, in_maps, core_ids=list(range(NCORES)),
                             trace=trace)
    if trace and r.exec_time_ns is not None:
        print(f"HW exec time: {r.exec_time_ns} ns")

    od = np.concatenate([r.results[c]["od_out"] for c in range(NCORES)],
                        axis=0)
    z = np.concatenate([r.results[c]["z_out"].T for c in range(NCORES)],
                       axis=0)
    return od, z


# revision 15
# speedup vs baseline: 1.4780x; 1.0480x over previous
"""GCN autoencoder (2-layer GCNConv encoder + inner-product decoder) on
8 Trainium2 NeuronCores.

Strategy (graph/data parallel, per the node-range sharding):
  - Nodes are sharded 2048/core by TARGET (col) range; edges are bucketed
    host-side by target tile (128 targets) and padded to a static block
    count so all 8 cores run one shared NEFF.
  - deg/dinv are computed on-device from a host-laid-out padded-CSR of
    edge weights (placement is host index work; all value math on device).
  - GCNConv is decomposed as out[c] = dinv[c] * sum_e w_e * G[row_e] + b
    with G[r] = dinv[r] * (x W)[r], so no per-edge dinv gathers are needed.
  - The scatter-add is a dense one-hot matmul: for each block of 128 edges
    (all targeting one 128-target tile), build onehot[e,t] = w_e*(col_e==t)
    with iota + tensor_scalar(is_equal, mult), gather the 128 source rows
    of G with one bulk dma_gather, and accumulate PSUM[t,f] += onehot.T @ G.
  - Full G is assembled between layers with an AllGather collective.
  - Decoder: zT ([64,16384]) lives in SBUF; od rows are sharded per core;
    softplus is Ln(1+Exp(x)) (both funcs in one ACT table).
"""

import os
import numpy as np
import ml_dtypes

import concourse.bass as bass
import concourse.tile as tile
from concourse import bacc, mybir
from concourse.bass_utils import run_bass_kernel_spmd
from concourse.masks import make_identity
from contextlib import ExitStack

P = 128
NCORES = 8
N = 16384
IN_DIM = 256
HID = 128
EMB = 64
E = 524288
NLOC = N // NCORES            # 2048 nodes per core
NT = NLOC // P                # 16 target tiles per core
TILES_PER_GATHER = 1          # gather granularity (tiles)

FP = mybir.dt.float32
BF = mybir.dt.bfloat16
I16 = mybir.dt.int16

_AF = mybir.ActivationFunctionType
_OP = mybir.AluOpType


# ----------------------------------------------------------------------
# host-side layout prep (index work only; all value math runs on device)
# ----------------------------------------------------------------------

def _prep_inputs(x, row, col, w):
    """Bucket edges by target tile, pad to a static block count, and build
    the per-core input arrays."""
    rows_all = np.concatenate([row, np.arange(N, dtype=np.int64)])
    cols_all = np.concatenate([col, np.arange(N, dtype=np.int64)])
    w_all = np.concatenate([w, np.ones(N, np.float32)]).astype(np.float32)

    tile_id = (cols_all // P).astype(np.int64)          # 0..127 global tiles
    order = np.argsort(tile_id, kind="stable")
    rows_s, cols_s, w_s = rows_all[order], cols_all[order], w_all[order]
    tile_s = tile_id[order]
    counts = np.bincount(tile_s, minlength=N // P)       # edges per tile
    NBT = int(np.ceil(counts.max() / P))                 # blocks per tile
    starts = np.concatenate([[0], np.cumsum(counts)])

    # per-target degree CSR width
    deg_counts = np.bincount(cols_all, minlength=N)
    DMAX = int(deg_counts.max())

    per_core = []
    NB_TOT = NT * NBT
    NI = NB_TOT * P
    for c in range(NCORES):
        idx_flat = np.zeros(NI, np.int64)
        colv = np.zeros((P, NB_TOT), np.float32)
        wv = np.zeros((P, NB_TOT), np.float32)
        wcsr = np.zeros((P, NT, DMAX), np.float32)
        for lt in range(NT):
            t = c * NT + lt
            s, e = starts[t], starts[t + 1]
            cnt = e - s
            base = lt * NBT * P
            idx_flat[base:base + cnt] = rows_s[s:e]
            cl = (cols_s[s:e] - t * P).astype(np.float32)
            ww = w_s[s:e]
            flat_cl = np.zeros(NBT * P, np.float32)
            flat_w = np.zeros(NBT * P, np.float32)
            flat_cl[:cnt] = cl
            flat_w[:cnt] = ww
            colv[:, lt * NBT:(lt + 1) * NBT] = flat_cl.reshape(NBT, P).T
            wv[:, lt * NBT:(lt + 1) * NBT] = flat_w.reshape(NBT, P).T
            # padded-CSR of weights per target for the degree reduce
            tgt_local = (cols_s[s:e] - t * P).astype(np.int64)
            slot = np.zeros(P, np.int64)
            for j in range(cnt):
                p = tgt_local[j]
                wcsr[p, lt, slot[p]] = ww[j]
                slot[p] += 1
        # dma_gather idx layout: unwrapped[i] = idxs[i%16, i//16],
        # replicated across the 8 groups of 16 partitions
        idx16 = np.zeros((16, NI // 16), np.int16)
        idx16[np.arange(NI) % 16, np.arange(NI) // 16] = idx_flat.astype(np.int16)
        idx_lay = np.tile(idx16, (8, 1))
        per_core.append(dict(idx=idx_lay,
                             colv=colv, wv=wv,
                             wcsr=wcsr.reshape(P, NT * DMAX)))
    return per_core, NBT, DMAX


# ----------------------------------------------------------------------
# device kernel
# ----------------------------------------------------------------------

def _build(NBT, DMAX):
    NB_TOT = NT * NBT
    NI = NB_TOT * P
    GB = TILES_PER_GATHER * NBT          # blocks per gather
    NIG = GB * P                         # idxs per gather
    NGATHER = NT // TILES_PER_GATHER

    nc = bacc.Bacc("TRN2", target_bir_lowering=False, debug=False,
                   num_devices=NCORES, num_swdge_queues=4)

    xT_in = nc.dram_tensor("xT_in", [IN_DIM, NLOC], FP, kind="ExternalInput").ap()
    w1_in = nc.dram_tensor("w1_in", [IN_DIM, HID], FP, kind="ExternalInput").ap()
    b1_in = nc.dram_tensor("b1_in", [1, HID], FP, kind="ExternalInput").ap()
    w2_in = nc.dram_tensor("w2_in", [HID, EMB], FP, kind="ExternalInput").ap()
    b2_in = nc.dram_tensor("b2_in", [EMB, 1], FP, kind="ExternalInput").ap()
    db_in = nc.dram_tensor("db_in", [1, 1], FP, kind="ExternalInput").ap()
    idx_in = nc.dram_tensor("idx_in", [P, NI // 16], I16, kind="ExternalInput").ap()
    col_in = nc.dram_tensor("col_in", [P, NB_TOT], FP, kind="ExternalInput").ap()
    wv_in = nc.dram_tensor("wv_in", [P, NB_TOT], FP, kind="ExternalInput").ap()
    wcsr_in = nc.dram_tensor("wcsr_in", [P, NT * DMAX], FP, kind="ExternalInput").ap()

    od_out = nc.dram_tensor("od_out", [NLOC, N], FP, kind="ExternalOutput").ap()
    z_out = nc.dram_tensor("z_out", [EMB, NLOC], FP, kind="ExternalOutput").ap()

    with tile.TileContext(nc) as tc, ExitStack() as ctx:
        dram = ctx.enter_context(tc.tile_pool(name="dram", bufs=1, space="DRAM"))
        g1loc = dram.tile([NLOC, HID], BF)
        g1full = dram.tile([N, HID], BF, addr_space="Shared")
        g2loc = dram.tile([NLOC, HID], BF)
        g2full = dram.tile([N, HID], BF, addr_space="Shared")
        ztloc = dram.tile([EMB, NLOC], FP)
        ztstack = dram.tile([NCORES * EMB, NLOC], FP, addr_space="Shared")

        # ---- persistent small tiles ---------------------------------
        const = ctx.enter_context(tc.tile_pool(name="const", bufs=1))
        iota = const.tile([P, P], FP)
        nc.gpsimd.iota(iota[:], pattern=[[1, P]], base=0, channel_multiplier=0,
                       allow_small_or_imprecise_dtypes=True)
        ident = const.tile([P, P], FP)
        make_identity(nc, ident[:])
        iotab = const.tile([P, P], BF)
        nc.vector.tensor_copy(iotab[:], iota[:])
        ones1 = const.tile([1, P], FP)
        nc.vector.memset(ones1[:], 1.0)

        w1sb = const.tile([P, 2, HID], FP)   # [k-half][kp, f1]
        nc.sync.dma_start(w1sb[:, 0, :], w1_in[0:P, :])
        nc.sync.dma_start(w1sb[:, 1, :], w1_in[P:IN_DIM, :])
        w2sb = const.tile([HID, EMB], FP)
        nc.sync.dma_start(w2sb[:], w2_in[:])
        b1sb = const.tile([1, HID], FP)
        nc.sync.dma_start(b1sb[:], b1_in[:])
        b2sb = const.tile([EMB, 1], FP)
        nc.sync.dma_start(b2sb[:], b2_in[:])
        dbsb = const.tile([1, 1], FP)
        nc.sync.dma_start(dbsb[:], db_in[:])

        colv = const.tile([P, NB_TOT], FP)
        nc.sync.dma_start(colv[:], col_in[:])
        wv = const.tile([P, NB_TOT], FP)
        nc.sync.dma_start(wv[:], wv_in[:])
        idxs = const.tile([P, NI // 16], I16)
        nc.sync.dma_start(idxs[:], idx_in[:])

        # ---- phase 1: degree -> dinv --------------------------------
        with tc.tile_pool(name="p1", bufs=1) as p1, \
             tc.tile_pool(name="p1ps", bufs=1, space="PSUM") as p1ps:
            wcsr = p1.tile([P, NT, DMAX], FP)
            nc.sync.dma_start(wcsr[:, :, :], wcsr_in[:].rearrange(
                "p (t d) -> p t d", t=NT))
            deg = const.tile([P, NT], FP)
            nc.vector.tensor_reduce(deg[:], wcsr[:, :, :],
                                    axis=mybir.AxisListType.X, op=_OP.add)
            sdeg = const.tile([P, NT], FP)
            nc.scalar.activation(sdeg[:], deg[:], _AF.Sqrt)
            dinv = const.tile([P, NT], FP)
            nc.vector.reciprocal(dinv[:], sdeg[:])
            # sdeg transposed: row t holds sqrt(deg) of tile t's targets
            sdT_ps = p1ps.tile([NT, P], FP, space="PSUM")
            nc.tensor.transpose(sdT_ps[:], sdeg[:], ident[:])
            sdegT = const.tile([NT, P], FP)
            nc.vector.tensor_copy(sdegT[:], sdT_ps[:])
            # matmul lhsT needs base partition 0: flatten the [NT, P] rows
            # into one [1, NT*P] row on partition 0, via a DRAM bounce
            sd_dram = dram.tile([NT, P], FP)
            nc.sync.dma_start(sd_dram[:], sdegT[:])
            sdegrows = const.tile([1, NT * P], FP)
            nc.sync.dma_start(sdegrows[:],
                              sd_dram[:].rearrange("(a t) p -> a (t p)", a=1))

        # decoder bias broadcast [128,1]
        with tc.tile_pool(name="p1b", bufs=1, space="PSUM") as p1b:
            db_ps = p1b.tile([P, 1], FP, space="PSUM")
            nc.tensor.matmul(db_ps[:], lhsT=ones1[:], rhs=dbsb[:],
                             start=True, stop=True)
            decb = const.tile([P, 1], FP)
            nc.vector.tensor_copy(decb[:], db_ps[:])

        # ---- phase 2: G1 = dinv * (x @ W1) for own nodes ------------
        with tc.tile_pool(name="p2", bufs=3) as p2, \
             tc.tile_pool(name="p2ps", bufs=3, space="PSUM") as p2ps:
            xt = p2.tile([P, 2, NLOC], FP, tag="xt", bufs=1)
            nc.sync.dma_start(xt[:, 0, :], xT_in[0:P, :])
            nc.sync.dma_start(xt[:, 1, :], xT_in[P:IN_DIM, :])
            for c in range(NT):
                hps = p2ps.tile([P, HID], FP, space="PSUM", tag="hps")
                nc.tensor.matmul(hps[:], lhsT=xt[:, 0, bass.ts(c, P)],
                                 rhs=w1sb[:, 0, :], start=True, stop=False)
                nc.tensor.matmul(hps[:], lhsT=xt[:, 1, bass.ts(c, P)],
                                 rhs=w1sb[:, 1, :], start=False, stop=True)
                g1t = p2.tile([P, HID], BF, tag="g1t")
                nc.scalar.activation(g1t[:], hps[:], _AF.Copy,
                                     scale=dinv[:, c:c + 1])
                nc.sync.dma_start(g1loc[bass.ts(c, P), :], g1t[:])
        nc.gpsimd.collective_compute(
            "AllGather", _OP.bypass, replica_groups=[list(range(NCORES))],
            ins=[g1loc[:].opt()], outs=[g1full[:].opt()])

        # ---- phases 3 & 4: the two aggregation layers ---------------
        def aggregate(gfull, layer):
            """One-hot matmul aggregation over all 16 target tiles.
            layer==1: emit G2 tiles -> g2loc. layer==2: emit zT -> ztloc."""
            with tc.tile_pool(name=f"ag{layer}", bufs=2) as ag, \
                 tc.tile_pool(name=f"ag{layer}ps", bufs=4, space="PSUM") as agps, \
                 tc.tile_pool(name=f"ag{layer}o", bufs=3) as ago:
                if layer == 2:
                    ztsb = ago.tile([EMB, NLOC], FP, tag="ztsb", bufs=1)
                for k in range(NGATHER):
                    gath = ag.tile([P, GB, HID], BF, tag="gath", bufs=4)
                    nc.gpsimd.dma_gather(
                        gath[:, :, :], gfull[:],
                        idxs[:, k * (NIG // 16):(k + 1) * (NIG // 16)],
                        NIG, NIG, HID, elem_step=HID,
                        # >1008 idxs overflows the 64-desc/engine packet
                        single_packet=False, queue_num=1 + (k % 3))
                    for lt in range(TILES_PER_GATHER):
                        t = k * TILES_PER_GATHER + lt
                        aps = agps.tile([P, HID], FP, space="PSUM", tag="aps",
                                        bufs=4)
                        for b in range(NBT):
                            gb = t * NBT + b
                            oh = ag.tile([P, P], BF, tag="oh", bufs=48)
                            nc.vector.tensor_scalar(
                                out=oh[:], in0=iotab[:],
                                scalar1=colv[:, gb:gb + 1],
                                scalar2=wv[:, gb:gb + 1],
                                op0=_OP.is_equal, op1=_OP.mult)
                            nc.tensor.matmul(
                                aps[:], lhsT=oh[:], rhs=gath[:, lt * NBT + b, :],
                                start=(b == 0),
                                stop=(layer == 2 and b == NBT - 1))
                        if layer == 1:
                            # += sqrt(deg)[t] (x) b1 so that
                            # relu(dinv*psum + b1) comes out of one ACT op
                            nc.tensor.matmul(
                                aps[:], lhsT=sdegrows[:, bass.ts(t, P)],
                                rhs=b1sb[:], start=False, stop=True)
                            h2 = ago.tile([P, HID], FP, tag="h2")
                            nc.scalar.activation(h2[:], aps[:], _AF.Relu,
                                                 scale=dinv[:, t:t + 1])
                            g2t = ago.tile([P, HID], BF, tag="g2t")
                            nc.vector.tensor_scalar_mul(g2t[:], h2[:],
                                                        dinv[:, t:t + 1])
                            nc.sync.dma_start(g2loc[bass.ts(t, P), :], g2t[:])
                        else:
                            a2d = ago.tile([P, HID], FP, tag="a2d")
                            nc.scalar.activation(a2d[:], aps[:], _AF.Copy,
                                                 scale=dinv[:, t:t + 1])
                            tps = agps.tile([P, P], FP, space="PSUM", tag="tps",
                                            bufs=2)
                            nc.tensor.transpose(tps[:], a2d[:], ident[:])
                            a2dT = ago.tile([P, P], FP, tag="a2dT")
                            nc.vector.tensor_copy(a2dT[:], tps[:])
                            zps = agps.tile([EMB, P], FP, space="PSUM", tag="zps",
                                            bufs=2)
                            nc.tensor.matmul(zps[:], lhsT=w2sb[:],
                                             rhs=a2dT[:], start=True, stop=True)
                            nc.scalar.activation(ztsb[:, bass.ts(t, P)], zps[:],
                                                 _AF.Identity, bias=b2sb[:, :1])
                if layer == 2:
                    nc.sync.dma_start(ztloc[:], ztsb[:])
                    nc.sync.dma_start(z_out[:], ztsb[:])

        aggregate(g1full, 1)
        nc.gpsimd.collective_compute(
            "AllGather", _OP.bypass, replica_groups=[list(range(NCORES))],
            ins=[g2loc[:].opt()], outs=[g2full[:].opt()])

        aggregate(g2full, 2)
        nc.gpsimd.collective_compute(
            "AllGather", _OP.bypass, replica_groups=[list(range(NCORES))],
            ins=[ztloc[:].opt()], outs=[ztstack[:].opt()])

        # ---- phase 5: decoder od = softplus(z @ z.T + db) -----------
        OCH = 8192            # output buffer width
        DCH = 2048            # psum / ACT chunk width
        with tc.tile_pool(name="p5", bufs=2) as p5, \
             tc.tile_pool(name="p5ps", bufs=2, space="PSUM") as p5ps:
            ztfull = p5.tile([EMB, N], BF, tag="ztfull", bufs=1)
            for d in range(NCORES):
                nc.gpsimd.dma_start(ztfull[:, bass.ts(d, NLOC)],
                                    ztstack[bass.ts(d, EMB), :])
            # this core's own zT rows for the stationary side
            ztmine = p5.tile([EMB, NLOC], BF, tag="ztmine", bufs=1)
            nc.gpsimd.dma_start(ztmine[:], ztloc[:])
            for m in range(NT):
                for h in range(N // OCH):
                    obuf = p5.tile([P, OCH], FP, tag="obuf")
                    tbuf = p5.tile([P, OCH], FP, tag="tbuf", bufs=1)
                    for q in range(OCH // DCH):
                        dps = p5ps.tile([P, DCH], FP, space="PSUM", tag="dps")
                        for s in range(DCH // 512):
                            n0 = h * OCH + q * DCH + s * 512
                            nc.tensor.matmul(
                                dps[:, bass.ts(s, 512)],
                                lhsT=ztmine[:, bass.ts(m, P)],
                                rhs=ztfull[:, n0:n0 + 512],
                                start=True, stop=True)
                        # batch all Exp ops, then one wide Ln: avoids
                        # per-op ACT table thrash (Exp and Ln live in
                        # different default table sets)
                        nc.scalar.activation(tbuf[:, bass.ts(q, DCH)], dps[:],
                                             _AF.Exp, bias=decb[:, :1])
                    nc.scalar.activation(obuf[:], tbuf[:], _AF.Ln, bias=1.0)
                    nc.sync.dma_start(
                        od_out[bass.ts(m, P), bass.ts(h, OCH)], obuf[:])

    nc.compile()
    return nc


_BUILD_CACHE = {}


def kernel(x, edge_index, edge_weight, W1, b1, W2, b2, dec_bias):
    x = np.asarray(x, np.float32)
    edge_index = np.asarray(edge_index, np.int64)
    edge_weight = np.asarray(edge_weight, np.float32)
    W1 = np.asarray(W1, np.float32)
    b1 = np.asarray(b1, np.float32)
    W2 = np.asarray(W2, np.float32)
    b2 = np.asarray(b2, np.float32)
    dec_bias = np.asarray(dec_bias, np.float32)

    per_core, NBT, DMAX = _prep_inputs(x, edge_index[0], edge_index[1],
                                       edge_weight)

    key = (NBT, DMAX)
    if key not in _BUILD_CACHE:
        _BUILD_CACHE[key] = _build(NBT, DMAX)
    nc = _BUILD_CACHE[key]

    in_maps = []
    for c in range(NCORES):
        pc = per_core[c]
        in_maps.append({
            "xT_in": np.ascontiguousarray(
                x[c * NLOC:(c + 1) * NLOC, :].T),
            "w1_in": W1,
            "b1_in": b1.reshape(1, HID),
            "w2_in": W2,
            "b2_in": b2.reshape(EMB, 1),
            "db_in": dec_bias.reshape(1, 1),
            "idx_in": pc["idx"],
            "col_in": pc["colv"],
            "wv_in": pc["wv"],
            "wcsr_in": pc["wcsr"],
        })

    trace = os.environ.get("GCN_TRACE") == "1"
    r = run_bass_kernel_spmd(nc, in_maps, core_ids=list(range(NCORES)),
                             trace=trace)
    if trace and r.exec_time_ns is not None:
        print(f"HW exec time: {r.exec_time_ns} ns")

    od = np.concatenate([r.results[c]["od_out"] for c in range(NCORES)],
                        axis=0)
    z = np.concatenate([r.results[c]["z_out"].T for c in range(NCORES)],
                       axis=0)
    return od, z


# revision 16
# speedup vs baseline: 1.6929x; 1.1454x over previous
"""GCN autoencoder (2-layer GCNConv encoder + inner-product decoder) on
8 Trainium2 NeuronCores.

Strategy (graph/data parallel, per the node-range sharding):
  - Nodes are sharded 2048/core by TARGET (col) range; edges are bucketed
    host-side by target tile (128 targets) and padded to a static block
    count so all 8 cores run one shared NEFF.
  - deg/dinv are computed on-device from a host-laid-out padded-CSR of
    edge weights (placement is host index work; all value math on device).
  - GCNConv is decomposed as out[c] = dinv[c] * sum_e w_e * G[row_e] + b
    with G[r] = dinv[r] * (x W)[r], so no per-edge dinv gathers are needed.
  - The scatter-add is a dense one-hot matmul: for each block of 128 edges
    (all targeting one 128-target tile), build onehot[e,t] = w_e*(col_e==t)
    with iota + tensor_scalar(is_equal, mult), gather the 128 source rows
    of G with one bulk dma_gather, and accumulate PSUM[t,f] += onehot.T @ G.
  - Full G is assembled between layers with an AllGather collective.
  - Decoder: zT ([64,16384]) lives in SBUF; od rows are sharded per core;
    softplus is Ln(1+Exp(x)) (both funcs in one ACT table).
"""

import os
import numpy as np
import ml_dtypes

import concourse.bass as bass
import concourse.tile as tile
from concourse import bacc, mybir
from concourse.bass_utils import run_bass_kernel_spmd
from concourse.masks import make_identity
from contextlib import ExitStack

P = 128
NCORES = 8
N = 16384
IN_DIM = 256
HID = 128
EMB = 64
E = 524288
NLOC = N // NCORES            # 2048 nodes per core
NT = NLOC // P                # 16 target tiles per core
TILES_PER_GATHER = 1          # gather granularity (tiles)

FP = mybir.dt.float32
BF = mybir.dt.bfloat16
I16 = mybir.dt.int16

_AF = mybir.ActivationFunctionType
_OP = mybir.AluOpType


# ----------------------------------------------------------------------
# host-side layout prep (index work only; all value math runs on device)
# ----------------------------------------------------------------------

def _prep_inputs(x, row, col, w):
    """Bucket edges by target tile, pad to a static block count, and build
    the per-core input arrays."""
    rows_all = np.concatenate([row, np.arange(N, dtype=np.int64)])
    cols_all = np.concatenate([col, np.arange(N, dtype=np.int64)])
    w_all = np.concatenate([w, np.ones(N, np.float32)]).astype(np.float32)

    tile_id = (cols_all // P).astype(np.int64)          # 0..127 global tiles
    order = np.argsort(tile_id, kind="stable")
    rows_s, cols_s, w_s = rows_all[order], cols_all[order], w_all[order]
    tile_s = tile_id[order]
    counts = np.bincount(tile_s, minlength=N // P)       # edges per tile
    NBT = int(np.ceil(counts.max() / P))                 # blocks per tile
    starts = np.concatenate([[0], np.cumsum(counts)])

    # per-target degree CSR width
    deg_counts = np.bincount(cols_all, minlength=N)
    DMAX = int(deg_counts.max())

    per_core = []
    NB_TOT = NT * NBT
    NI = NB_TOT * P
    for c in range(NCORES):
        idx_flat = np.zeros(NI, np.int64)
        colv = np.zeros((P, NB_TOT), np.float32)
        wv = np.zeros((P, NB_TOT), np.float32)
        wcsr = np.zeros((P, NT, DMAX), np.float32)
        for lt in range(NT):
            t = c * NT + lt
            s, e = starts[t], starts[t + 1]
            cnt = e - s
            base = lt * NBT * P
            idx_flat[base:base + cnt] = rows_s[s:e]
            cl = (cols_s[s:e] - t * P).astype(np.float32)
            ww = w_s[s:e]
            flat_cl = np.zeros(NBT * P, np.float32)
            flat_w = np.zeros(NBT * P, np.float32)
            flat_cl[:cnt] = cl
            flat_w[:cnt] = ww
            colv[:, lt * NBT:(lt + 1) * NBT] = flat_cl.reshape(NBT, P).T
            wv[:, lt * NBT:(lt + 1) * NBT] = flat_w.reshape(NBT, P).T
            # padded-CSR of weights per target for the degree reduce
            tgt_local = (cols_s[s:e] - t * P).astype(np.int64)
            slot = np.zeros(P, np.int64)
            for j in range(cnt):
                p = tgt_local[j]
                wcsr[p, lt, slot[p]] = ww[j]
                slot[p] += 1
        # dma_gather idx layout: unwrapped[i] = idxs[i%16, i//16],
        # replicated across the 8 groups of 16 partitions
        idx16 = np.zeros((16, NI // 16), np.int16)
        idx16[np.arange(NI) % 16, np.arange(NI) // 16] = idx_flat.astype(np.int16)
        idx_lay = np.tile(idx16, (8, 1))
        per_core.append(dict(idx=idx_lay,
                             colv=colv, wv=wv,
                             wcsr=wcsr.reshape(P, NT * DMAX)))
    return per_core, NBT, DMAX


# ----------------------------------------------------------------------
# device kernel
# ----------------------------------------------------------------------

def _build(NBT, DMAX):
    NB_TOT = NT * NBT
    NI = NB_TOT * P
    GB = TILES_PER_GATHER * NBT          # blocks per gather
    NIG = GB * P                         # idxs per gather
    NGATHER = NT // TILES_PER_GATHER

    nc = bacc.Bacc("TRN2", target_bir_lowering=False, debug=False,
                   num_devices=NCORES, num_swdge_queues=4)

    xT_in = nc.dram_tensor("xT_in", [IN_DIM, NLOC], FP, kind="ExternalInput").ap()
    w1_in = nc.dram_tensor("w1_in", [IN_DIM, HID], FP, kind="ExternalInput").ap()
    b1_in = nc.dram_tensor("b1_in", [1, HID], FP, kind="ExternalInput").ap()
    w2_in = nc.dram_tensor("w2_in", [HID, EMB], FP, kind="ExternalInput").ap()
    b2_in = nc.dram_tensor("b2_in", [EMB, 1], FP, kind="ExternalInput").ap()
    db_in = nc.dram_tensor("db_in", [1, 1], FP, kind="ExternalInput").ap()
    idx_in = nc.dram_tensor("idx_in", [P, NI // 16], I16, kind="ExternalInput").ap()
    col_in = nc.dram_tensor("col_in", [P, NB_TOT], FP, kind="ExternalInput").ap()
    wv_in = nc.dram_tensor("wv_in", [P, NB_TOT], FP, kind="ExternalInput").ap()
    wcsr_in = nc.dram_tensor("wcsr_in", [P, NT * DMAX], FP, kind="ExternalInput").ap()

    od_out = nc.dram_tensor("od_out", [NLOC, N], FP, kind="ExternalOutput").ap()
    z_out = nc.dram_tensor("z_out", [EMB, NLOC], FP, kind="ExternalOutput").ap()

    with tile.TileContext(nc) as tc, ExitStack() as ctx:
        dram = ctx.enter_context(tc.tile_pool(name="dram", bufs=1, space="DRAM"))
        g1loc = dram.tile([NLOC, HID], BF)
        g1full = dram.tile([N, HID], BF, addr_space="Shared")
        g2loc = dram.tile([NLOC, HID], BF)
        g2full = dram.tile([N, HID], BF, addr_space="Shared")
        ztloc = dram.tile([EMB, NLOC], FP)
        ztstack = dram.tile([NCORES * EMB, NLOC], FP, addr_space="Shared")

        # ---- persistent small tiles ---------------------------------
        const = ctx.enter_context(tc.tile_pool(name="const", bufs=1))
        iota = const.tile([P, P], FP)
        nc.gpsimd.iota(iota[:], pattern=[[1, P]], base=0, channel_multiplier=0,
                       allow_small_or_imprecise_dtypes=True)
        ident = const.tile([P, P], FP)
        make_identity(nc, ident[:])
        iotab = const.tile([P, P], BF)
        nc.vector.tensor_copy(iotab[:], iota[:])
        ones1 = const.tile([1, P], FP)
        nc.vector.memset(ones1[:], 1.0)

        w1sb = const.tile([P, 2, HID], FP)   # [k-half][kp, f1]
        nc.sync.dma_start(w1sb[:, 0, :], w1_in[0:P, :])
        nc.sync.dma_start(w1sb[:, 1, :], w1_in[P:IN_DIM, :])
        w2sb = const.tile([HID, EMB], FP)
        nc.sync.dma_start(w2sb[:], w2_in[:])
        b1sb = const.tile([1, HID], FP)
        nc.sync.dma_start(b1sb[:], b1_in[:])
        b2sb = const.tile([EMB, 1], FP)
        nc.sync.dma_start(b2sb[:], b2_in[:])
        dbsb = const.tile([1, 1], FP)
        nc.sync.dma_start(dbsb[:], db_in[:])

        colv = const.tile([P, NB_TOT], FP)
        nc.sync.dma_start(colv[:], col_in[:])
        wv = const.tile([P, NB_TOT], FP)
        nc.sync.dma_start(wv[:], wv_in[:])
        idxs = const.tile([P, NI // 16], I16)
        nc.sync.dma_start(idxs[:], idx_in[:])

        # ---- phase 1: degree -> dinv --------------------------------
        with tc.tile_pool(name="p1", bufs=1) as p1, \
             tc.tile_pool(name="p1ps", bufs=1, space="PSUM") as p1ps:
            wcsr = p1.tile([P, NT, DMAX], FP)
            nc.sync.dma_start(wcsr[:, :, :], wcsr_in[:].rearrange(
                "p (t d) -> p t d", t=NT))
            deg = const.tile([P, NT], FP)
            nc.vector.tensor_reduce(deg[:], wcsr[:, :, :],
                                    axis=mybir.AxisListType.X, op=_OP.add)
            sdeg = const.tile([P, NT], FP)
            nc.scalar.activation(sdeg[:], deg[:], _AF.Sqrt)
            dinv = const.tile([P, NT], FP)
            nc.vector.reciprocal(dinv[:], sdeg[:])
            # sdeg transposed: row t holds sqrt(deg) of tile t's targets
            sdT_ps = p1ps.tile([NT, P], FP, space="PSUM")
            nc.tensor.transpose(sdT_ps[:], sdeg[:], ident[:])
            sdegT = const.tile([NT, P], FP)
            nc.vector.tensor_copy(sdegT[:], sdT_ps[:])
            # matmul lhsT needs base partition 0: flatten the [NT, P] rows
            # into one [1, NT*P] row on partition 0, via a DRAM bounce
            sd_dram = dram.tile([NT, P], FP)
            nc.sync.dma_start(sd_dram[:], sdegT[:])
            sdegrows = const.tile([1, NT * P], FP)
            nc.sync.dma_start(sdegrows[:],
                              sd_dram[:].rearrange("(a t) p -> a (t p)", a=1))

        # decoder bias broadcast [128,1]
        with tc.tile_pool(name="p1b", bufs=1, space="PSUM") as p1b:
            db_ps = p1b.tile([P, 1], FP, space="PSUM")
            nc.tensor.matmul(db_ps[:], lhsT=ones1[:], rhs=dbsb[:],
                             start=True, stop=True)
            decb = const.tile([P, 1], FP)
            nc.vector.tensor_copy(decb[:], db_ps[:])

        # ---- phase 2: G1 = dinv * (x @ W1) for own nodes ------------
        with tc.tile_pool(name="p2", bufs=3) as p2, \
             tc.tile_pool(name="p2ps", bufs=3, space="PSUM") as p2ps:
            xt = p2.tile([P, 2, NLOC], FP, tag="xt", bufs=1)
            nc.sync.dma_start(xt[:, 0, :], xT_in[0:P, :])
            nc.sync.dma_start(xt[:, 1, :], xT_in[P:IN_DIM, :])
            for c in range(NT):
                hps = p2ps.tile([P, HID], FP, space="PSUM", tag="hps")
                nc.tensor.matmul(hps[:], lhsT=xt[:, 0, bass.ts(c, P)],
                                 rhs=w1sb[:, 0, :], start=True, stop=False)
                nc.tensor.matmul(hps[:], lhsT=xt[:, 1, bass.ts(c, P)],
                                 rhs=w1sb[:, 1, :], start=False, stop=True)
                g1t = p2.tile([P, HID], BF, tag="g1t")
                nc.scalar.activation(g1t[:], hps[:], _AF.Copy,
                                     scale=dinv[:, c:c + 1])
                nc.sync.dma_start(g1loc[bass.ts(c, P), :], g1t[:])
        nc.gpsimd.collective_compute(
            "AllGather", _OP.bypass, replica_groups=[list(range(NCORES))],
            ins=[g1loc[:].opt()], outs=[g1full[:].opt()])

        # ---- phases 3 & 4: the two aggregation layers ---------------
        def aggregate(gfull, layer):
            """One-hot matmul aggregation over all 16 target tiles.
            layer==1: emit G2 tiles -> g2loc. layer==2: emit zT -> ztloc."""
            with tc.tile_pool(name=f"ag{layer}", bufs=2) as ag, \
                 tc.tile_pool(name=f"ag{layer}ps", bufs=4, space="PSUM") as agps, \
                 tc.tile_pool(name=f"ag{layer}o", bufs=3) as ago:
                if layer == 2:
                    ztsb = ago.tile([EMB, NLOC], FP, tag="ztsb", bufs=1)
                for k in range(NGATHER):
                    gath = ag.tile([P, GB, HID], BF, tag="gath", bufs=8)
                    nc.gpsimd.dma_gather(
                        gath[:, :, :], gfull[:],
                        idxs[:, k * (NIG // 16):(k + 1) * (NIG // 16)],
                        NIG, NIG, HID, elem_step=HID,
                        # >1008 idxs overflows the 64-desc/engine packet
                        single_packet=False, queue_num=1 + (k % 3))
                    for lt in range(TILES_PER_GATHER):
                        t = k * TILES_PER_GATHER + lt
                        aps = agps.tile([P, HID], FP, space="PSUM", tag="aps",
                                        bufs=4)
                        for b in range(NBT):
                            gb = t * NBT + b
                            oh = ag.tile([P, P], BF, tag="oh", bufs=48)
                            nc.vector.tensor_scalar(
                                out=oh[:], in0=iotab[:],
                                scalar1=colv[:, gb:gb + 1],
                                scalar2=wv[:, gb:gb + 1],
                                op0=_OP.is_equal, op1=_OP.mult)
                            nc.tensor.matmul(
                                aps[:], lhsT=oh[:], rhs=gath[:, lt * NBT + b, :],
                                start=(b == 0),
                                stop=(layer == 2 and b == NBT - 1))
                        if layer == 1:
                            # += sqrt(deg)[t] (x) b1 so that
                            # relu(dinv*psum + b1) comes out of one ACT op
                            nc.tensor.matmul(
                                aps[:], lhsT=sdegrows[:, bass.ts(t, P)],
                                rhs=b1sb[:], start=False, stop=True)
                            h2 = ago.tile([P, HID], FP, tag="h2")
                            nc.scalar.activation(h2[:], aps[:], _AF.Relu,
                                                 scale=dinv[:, t:t + 1])
                            g2t = ago.tile([P, HID], BF, tag="g2t")
                            nc.vector.tensor_scalar_mul(g2t[:], h2[:],
                                                        dinv[:, t:t + 1])
                            nc.sync.dma_start(g2loc[bass.ts(t, P), :], g2t[:])
                        else:
                            a2d = ago.tile([P, HID], FP, tag="a2d")
                            nc.scalar.activation(a2d[:], aps[:], _AF.Copy,
                                                 scale=dinv[:, t:t + 1])
                            tps = agps.tile([P, P], FP, space="PSUM", tag="tps",
                                            bufs=2)
                            nc.tensor.transpose(tps[:], a2d[:], ident[:])
                            a2dT = ago.tile([P, P], FP, tag="a2dT")
                            nc.vector.tensor_copy(a2dT[:], tps[:])
                            zps = agps.tile([EMB, P], FP, space="PSUM", tag="zps",
                                            bufs=2)
                            nc.tensor.matmul(zps[:], lhsT=w2sb[:],
                                             rhs=a2dT[:], start=True, stop=True)
                            nc.scalar.activation(ztsb[:, bass.ts(t, P)], zps[:],
                                                 _AF.Identity, bias=b2sb[:, :1])
                if layer == 2:
                    nc.sync.dma_start(ztloc[:], ztsb[:])
                    nc.sync.dma_start(z_out[:], ztsb[:])

        aggregate(g1full, 1)
        nc.gpsimd.collective_compute(
            "AllGather", _OP.bypass, replica_groups=[list(range(NCORES))],
            ins=[g2loc[:].opt()], outs=[g2full[:].opt()])

        aggregate(g2full, 2)
        nc.gpsimd.collective_compute(
            "AllGather", _OP.bypass, replica_groups=[list(range(NCORES))],
            ins=[ztloc[:].opt()], outs=[ztstack[:].opt()])

        # ---- phase 5: decoder od = softplus(z @ z.T + db) -----------
        OCH = 8192            # output buffer width
        DCH = 2048            # psum / ACT chunk width
        with tc.tile_pool(name="p5", bufs=2) as p5, \
             tc.tile_pool(name="p5ps", bufs=2, space="PSUM") as p5ps:
            ztfull = p5.tile([EMB, N], BF, tag="ztfull", bufs=1)
            for d in range(NCORES):
                nc.gpsimd.dma_start(ztfull[:, bass.ts(d, NLOC)],
                                    ztstack[bass.ts(d, EMB), :])
            # this core's own zT rows for the stationary side
            ztmine = p5.tile([EMB, NLOC], BF, tag="ztmine", bufs=1)
            nc.gpsimd.dma_start(ztmine[:], ztloc[:])
            for m in range(NT):
                for h in range(N // OCH):
                    obuf = p5.tile([P, OCH], FP, tag="obuf")
                    tbuf = p5.tile([P, OCH], FP, tag="tbuf", bufs=1)
                    for q in range(OCH // DCH):
                        dps = p5ps.tile([P, DCH], FP, space="PSUM", tag="dps")
                        for s in range(DCH // 512):
                            n0 = h * OCH + q * DCH + s * 512
                            nc.tensor.matmul(
                                dps[:, bass.ts(s, 512)],
                                lhsT=ztmine[:, bass.ts(m, P)],
                                rhs=ztfull[:, n0:n0 + 512],
                                start=True, stop=True)
                        # batch all Exp ops, then one wide Ln: avoids
                        # per-op ACT table thrash (Exp and Ln live in
                        # different default table sets)
                        nc.scalar.activation(tbuf[:, bass.ts(q, DCH)], dps[:],
                                             _AF.Exp, bias=decb[:, :1])
                    nc.scalar.activation(obuf[:], tbuf[:], _AF.Ln, bias=1.0)
                    nc.sync.dma_start(
                        od_out[bass.ts(m, P), bass.ts(h, OCH)], obuf[:])

    nc.compile()
    return nc


_BUILD_CACHE = {}


def kernel(x, edge_index, edge_weight, W1, b1, W2, b2, dec_bias):
    x = np.asarray(x, np.float32)
    edge_index = np.asarray(edge_index, np.int64)
    edge_weight = np.asarray(edge_weight, np.float32)
    W1 = np.asarray(W1, np.float32)
    b1 = np.asarray(b1, np.float32)
    W2 = np.asarray(W2, np.float32)
    b2 = np.asarray(b2, np.float32)
    dec_bias = np.asarray(dec_bias, np.float32)

    per_core, NBT, DMAX = _prep_inputs(x, edge_index[0], edge_index[1],
                                       edge_weight)

    key = (NBT, DMAX)
    if key not in _BUILD_CACHE:
        _BUILD_CACHE[key] = _build(NBT, DMAX)
    nc = _BUILD_CACHE[key]

    in_maps = []
    for c in range(NCORES):
        pc = per_core[c]
        in_maps.append({
            "xT_in": np.ascontiguousarray(
                x[c * NLOC:(c + 1) * NLOC, :].T),
            "w1_in": W1,
            "b1_in": b1.reshape(1, HID),
            "w2_in": W2,
            "b2_in": b2.reshape(EMB, 1),
            "db_in": dec_bias.reshape(1, 1),
            "idx_in": pc["idx"],
            "col_in": pc["colv"],
            "wv_in": pc["wv"],
            "wcsr_in": pc["wcsr"],
        })

    trace = os.environ.get("GCN_TRACE") == "1"
    r = run_bass_kernel_spmd(nc, in_maps, core_ids=list(range(NCORES)),
                             trace=trace)
    if trace and r.exec_time_ns is not None:
        print(f"HW exec time: {r.exec_time_ns} ns")

    od = np.concatenate([r.results[c]["od_out"] for c in range(NCORES)],
                        axis=0)
    z = np.concatenate([r.results[c]["z_out"].T for c in range(NCORES)],
                       axis=0)
    return od, z


# revision 17
# speedup vs baseline: 1.8301x; 1.0811x over previous
"""GCN autoencoder (2-layer GCNConv encoder + inner-product decoder) on
8 Trainium2 NeuronCores.

Strategy (graph/data parallel, per the node-range sharding):
  - Nodes are sharded 2048/core by TARGET (col) range; edges are bucketed
    host-side by target tile (128 targets) and padded to a static block
    count so all 8 cores run one shared NEFF.
  - deg/dinv are computed on-device from a host-laid-out padded-CSR of
    edge weights (placement is host index work; all value math on device).
  - GCNConv is decomposed as out[c] = dinv[c] * sum_e w_e * G[row_e] + b
    with G[r] = dinv[r] * (x W)[r], so no per-edge dinv gathers are needed.
  - The scatter-add is a dense one-hot matmul: for each block of 128 edges
    (all targeting one 128-target tile), build onehot[e,t] = w_e*(col_e==t)
    with iota + tensor_scalar(is_equal, mult), gather the 128 source rows
    of G with one bulk dma_gather, and accumulate PSUM[t,f] += onehot.T @ G.
  - Full G is assembled between layers with an AllGather collective.
  - Decoder: zT ([64,16384]) lives in SBUF; od rows are sharded per core;
    softplus is Ln(1+Exp(x)) (both funcs in one ACT table).
"""

import os
import numpy as np
import ml_dtypes

import concourse.bass as bass
import concourse.tile as tile
from concourse import bacc, mybir
from concourse.bass_utils import run_bass_kernel_spmd
from concourse.masks import make_identity
from contextlib import ExitStack

P = 128
NCORES = 8
N = 16384
IN_DIM = 256
HID = 128
EMB = 64
E = 524288
NLOC = N // NCORES            # 2048 nodes per core
NT = NLOC // P                # 16 target tiles per core
TILES_PER_GATHER = 1          # gather granularity (tiles)

FP = mybir.dt.float32
BF = mybir.dt.bfloat16
I16 = mybir.dt.int16

_AF = mybir.ActivationFunctionType
_OP = mybir.AluOpType


# ----------------------------------------------------------------------
# host-side layout prep (index work only; all value math runs on device)
# ----------------------------------------------------------------------

def _prep_inputs(x, row, col, w):
    """Bucket edges by target tile, pad to a static block count, and build
    the per-core input arrays."""
    rows_all = np.concatenate([row, np.arange(N, dtype=np.int64)])
    cols_all = np.concatenate([col, np.arange(N, dtype=np.int64)])
    w_all = np.concatenate([w, np.ones(N, np.float32)]).astype(np.float32)

    tile_id = (cols_all // P).astype(np.int64)          # 0..127 global tiles
    order = np.argsort(tile_id, kind="stable")
    rows_s, cols_s, w_s = rows_all[order], cols_all[order], w_all[order]
    tile_s = tile_id[order]
    counts = np.bincount(tile_s, minlength=N // P)       # edges per tile
    NBT = int(np.ceil(counts.max() / P))                 # blocks per tile
    starts = np.concatenate([[0], np.cumsum(counts)])

    # per-target degree CSR width
    deg_counts = np.bincount(cols_all, minlength=N)
    DMAX = int(deg_counts.max())

    per_core = []
    NB_TOT = NT * NBT
    NI = NB_TOT * P
    for c in range(NCORES):
        idx_flat = np.zeros(NI, np.int64)
        colv = np.zeros((P, NB_TOT), np.float32)
        wv = np.zeros((P, NB_TOT), np.float32)
        wcsr = np.zeros((P, NT, DMAX), np.float32)
        for lt in range(NT):
            t = c * NT + lt
            s, e = starts[t], starts[t + 1]
            cnt = e - s
            base = lt * NBT * P
            idx_flat[base:base + cnt] = rows_s[s:e]
            cl = (cols_s[s:e] - t * P).astype(np.float32)
            ww = w_s[s:e]
            flat_cl = np.zeros(NBT * P, np.float32)
            flat_w = np.zeros(NBT * P, np.float32)
            flat_cl[:cnt] = cl
            flat_w[:cnt] = ww
            colv[:, lt * NBT:(lt + 1) * NBT] = flat_cl.reshape(NBT, P).T
            wv[:, lt * NBT:(lt + 1) * NBT] = flat_w.reshape(NBT, P).T
            # padded-CSR of weights per target for the degree reduce
            tgt_local = (cols_s[s:e] - t * P).astype(np.int64)
            slot = np.zeros(P, np.int64)
            for j in range(cnt):
                p = tgt_local[j]
                wcsr[p, lt, slot[p]] = ww[j]
                slot[p] += 1
        # dma_gather idx layout: unwrapped[i] = idxs[i%16, i//16],
        # replicated across the 8 groups of 16 partitions
        idx16 = np.zeros((16, NI // 16), np.int16)
        idx16[np.arange(NI) % 16, np.arange(NI) // 16] = idx_flat.astype(np.int16)
        idx_lay = np.tile(idx16, (8, 1))
        per_core.append(dict(idx=idx_lay,
                             colv=colv, wv=wv,
                             wcsr=wcsr.reshape(P, NT * DMAX)))
    return per_core, NBT, DMAX


# ----------------------------------------------------------------------
# device kernel
# ----------------------------------------------------------------------

def _build(NBT, DMAX):
    NB_TOT = NT * NBT
    NI = NB_TOT * P
    GB = TILES_PER_GATHER * NBT          # blocks per gather
    NIG = GB * P                         # idxs per gather
    NGATHER = NT // TILES_PER_GATHER

    nc = bacc.Bacc("TRN2", target_bir_lowering=False, debug=False,
                   num_devices=NCORES, num_swdge_queues=4)

    xT_in = nc.dram_tensor("xT_in", [IN_DIM, NLOC], FP, kind="ExternalInput").ap()
    w1_in = nc.dram_tensor("w1_in", [IN_DIM, HID], FP, kind="ExternalInput").ap()
    b1_in = nc.dram_tensor("b1_in", [1, HID], FP, kind="ExternalInput").ap()
    w2_in = nc.dram_tensor("w2_in", [HID, EMB], FP, kind="ExternalInput").ap()
    b2_in = nc.dram_tensor("b2_in", [EMB, 1], FP, kind="ExternalInput").ap()
    db_in = nc.dram_tensor("db_in", [1, 1], FP, kind="ExternalInput").ap()
    idx_in = nc.dram_tensor("idx_in", [P, NI // 16], I16, kind="ExternalInput").ap()
    col_in = nc.dram_tensor("col_in", [P, NB_TOT], FP, kind="ExternalInput").ap()
    wv_in = nc.dram_tensor("wv_in", [P, NB_TOT], FP, kind="ExternalInput").ap()
    wcsr_in = nc.dram_tensor("wcsr_in", [P, NT * DMAX], FP, kind="ExternalInput").ap()

    od_out = nc.dram_tensor("od_out", [NLOC, N], FP, kind="ExternalOutput").ap()
    z_out = nc.dram_tensor("z_out", [EMB, NLOC], FP, kind="ExternalOutput").ap()

    with tile.TileContext(nc) as tc, ExitStack() as ctx:
        dram = ctx.enter_context(tc.tile_pool(name="dram", bufs=1, space="DRAM"))
        g1loc = dram.tile([NLOC, HID], BF)
        g1full = dram.tile([N, HID], BF, addr_space="Shared")
        g2loc = dram.tile([NLOC, HID], BF)
        g2full = dram.tile([N, HID], BF, addr_space="Shared")
        ztloc = dram.tile([EMB, NLOC], FP)
        ztstack = dram.tile([NCORES * EMB, NLOC], FP, addr_space="Shared")

        # ---- persistent small tiles ---------------------------------
        const = ctx.enter_context(tc.tile_pool(name="const", bufs=1))
        iota = const.tile([P, P], FP)
        nc.gpsimd.iota(iota[:], pattern=[[1, P]], base=0, channel_multiplier=0,
                       allow_small_or_imprecise_dtypes=True)
        ident = const.tile([P, P], FP)
        make_identity(nc, ident[:])
        iotab = const.tile([P, P], BF)
        nc.vector.tensor_copy(iotab[:], iota[:])
        ones1 = const.tile([1, P], FP)
        nc.vector.memset(ones1[:], 1.0)

        w1sb = const.tile([P, 2, HID], FP)   # [k-half][kp, f1]
        nc.sync.dma_start(w1sb[:, 0, :], w1_in[0:P, :])
        nc.sync.dma_start(w1sb[:, 1, :], w1_in[P:IN_DIM, :])
        w2sb = const.tile([HID, EMB], FP)
        nc.sync.dma_start(w2sb[:], w2_in[:])
        b1sb = const.tile([1, HID], FP)
        nc.sync.dma_start(b1sb[:], b1_in[:])
        b2sb = const.tile([EMB, 1], FP)
        nc.sync.dma_start(b2sb[:], b2_in[:])
        dbsb = const.tile([1, 1], FP)
        nc.sync.dma_start(dbsb[:], db_in[:])

        colv = const.tile([P, NB_TOT], FP)
        nc.sync.dma_start(colv[:], col_in[:])
        wv = const.tile([P, NB_TOT], FP)
        nc.sync.dma_start(wv[:], wv_in[:])
        idxs = const.tile([P, NI // 16], I16)
        nc.sync.dma_start(idxs[:], idx_in[:])

        # ---- phase 1: degree -> dinv --------------------------------
        with tc.tile_pool(name="p1", bufs=1) as p1, \
             tc.tile_pool(name="p1ps", bufs=1, space="PSUM") as p1ps:
            wcsr = p1.tile([P, NT, DMAX], FP)
            nc.sync.dma_start(wcsr[:, :, :], wcsr_in[:].rearrange(
                "p (t d) -> p t d", t=NT))
            deg = const.tile([P, NT], FP)
            nc.vector.tensor_reduce(deg[:], wcsr[:, :, :],
                                    axis=mybir.AxisListType.X, op=_OP.add)
            sdeg = const.tile([P, NT], FP)
            nc.scalar.activation(sdeg[:], deg[:], _AF.Sqrt)
            dinv = const.tile([P, NT], FP)
            nc.vector.reciprocal(dinv[:], sdeg[:])
            # sdeg transposed: row t holds sqrt(deg) of tile t's targets
            sdT_ps = p1ps.tile([NT, P], FP, space="PSUM")
            nc.tensor.transpose(sdT_ps[:], sdeg[:], ident[:])
            sdegT = const.tile([NT, P], FP)
            nc.vector.tensor_copy(sdegT[:], sdT_ps[:])
            # matmul lhsT needs base partition 0: flatten the [NT, P] rows
            # into one [1, NT*P] row on partition 0, via a DRAM bounce
            sd_dram = dram.tile([NT, P], FP)
            nc.sync.dma_start(sd_dram[:], sdegT[:])
            sdegrows = const.tile([1, NT * P], FP)
            nc.sync.dma_start(sdegrows[:],
                              sd_dram[:].rearrange("(a t) p -> a (t p)", a=1))

        # decoder bias broadcast [128,1]
        with tc.tile_pool(name="p1b", bufs=1, space="PSUM") as p1b:
            db_ps = p1b.tile([P, 1], FP, space="PSUM")
            nc.tensor.matmul(db_ps[:], lhsT=ones1[:], rhs=dbsb[:],
                             start=True, stop=True)
            decb = const.tile([P, 1], FP)
            nc.vector.tensor_copy(decb[:], db_ps[:])

        # ---- phase 2: G1 = dinv * (x @ W1) for own nodes ------------
        with tc.tile_pool(name="p2", bufs=3) as p2, \
             tc.tile_pool(name="p2ps", bufs=3, space="PSUM") as p2ps:
            xt = p2.tile([P, 2, NLOC], FP, tag="xt", bufs=1)
            nc.sync.dma_start(xt[:, 0, :], xT_in[0:P, :])
            nc.sync.dma_start(xt[:, 1, :], xT_in[P:IN_DIM, :])
            for c in range(NT):
                hps = p2ps.tile([P, HID], FP, space="PSUM", tag="hps")
                nc.tensor.matmul(hps[:], lhsT=xt[:, 0, bass.ts(c, P)],
                                 rhs=w1sb[:, 0, :], start=True, stop=False)
                nc.tensor.matmul(hps[:], lhsT=xt[:, 1, bass.ts(c, P)],
                                 rhs=w1sb[:, 1, :], start=False, stop=True)
                g1t = p2.tile([P, HID], BF, tag="g1t")
                nc.scalar.activation(g1t[:], hps[:], _AF.Copy,
                                     scale=dinv[:, c:c + 1])
                nc.sync.dma_start(g1loc[bass.ts(c, P), :], g1t[:])
        nc.gpsimd.collective_compute(
            "AllGather", _OP.bypass, replica_groups=[list(range(NCORES))],
            ins=[g1loc[:].opt()], outs=[g1full[:].opt()])

        # ---- phases 3 & 4: the two aggregation layers ---------------
        def aggregate(gfull, layer):
            """One-hot matmul aggregation over all 16 target tiles.
            layer==1: emit G2 tiles -> g2loc. layer==2: emit zT -> ztloc."""
            with tc.tile_pool(name=f"ag{layer}", bufs=2) as ag, \
                 tc.tile_pool(name=f"ag{layer}ps", bufs=4, space="PSUM") as agps, \
                 tc.tile_pool(name=f"ag{layer}o", bufs=3) as ago:
                if layer == 2:
                    ztsb = ago.tile([EMB, NLOC], FP, tag="ztsb", bufs=1)
                for k in range(NGATHER):
                    gath = ag.tile([P, GB, HID], BF, tag="gath", bufs=12)
                    nc.gpsimd.dma_gather(
                        gath[:, :, :], gfull[:],
                        idxs[:, k * (NIG // 16):(k + 1) * (NIG // 16)],
                        NIG, NIG, HID, elem_step=HID,
                        # >1008 idxs overflows the 64-desc/engine packet
                        single_packet=False, queue_num=1 + (k % 3))
                    for lt in range(TILES_PER_GATHER):
                        t = k * TILES_PER_GATHER + lt
                        aps = agps.tile([P, HID], FP, space="PSUM", tag="aps",
                                        bufs=4)
                        for b in range(NBT):
                            gb = t * NBT + b
                            oh = ag.tile([P, P], BF, tag="oh", bufs=48)
                            nc.vector.tensor_scalar(
                                out=oh[:], in0=iotab[:],
                                scalar1=colv[:, gb:gb + 1],
                                scalar2=wv[:, gb:gb + 1],
                                op0=_OP.is_equal, op1=_OP.mult)
                            nc.tensor.matmul(
                                aps[:], lhsT=oh[:], rhs=gath[:, lt * NBT + b, :],
                                start=(b == 0),
                                stop=(layer == 2 and b == NBT - 1))
                        if layer == 1:
                            # += sqrt(deg)[t] (x) b1 so that
                            # relu(dinv*psum + b1) comes out of one ACT op
                            nc.tensor.matmul(
                                aps[:], lhsT=sdegrows[:, bass.ts(t, P)],
                                rhs=b1sb[:], start=False, stop=True)
                            h2 = ago.tile([P, HID], FP, tag="h2")
                            nc.scalar.activation(h2[:], aps[:], _AF.Relu,
                                                 scale=dinv[:, t:t + 1])
                            g2t = ago.tile([P, HID], BF, tag="g2t")
                            nc.vector.tensor_scalar_mul(g2t[:], h2[:],
                                                        dinv[:, t:t + 1])
                            nc.sync.dma_start(g2loc[bass.ts(t, P), :], g2t[:])
                        else:
                            a2d = ago.tile([P, HID], FP, tag="a2d")
                            nc.scalar.activation(a2d[:], aps[:], _AF.Copy,
                                                 scale=dinv[:, t:t + 1])
                            tps = agps.tile([P, P], FP, space="PSUM", tag="tps",
                                            bufs=2)
                            nc.tensor.transpose(tps[:], a2d[:], ident[:])
                            a2dT = ago.tile([P, P], FP, tag="a2dT")
                            nc.vector.tensor_copy(a2dT[:], tps[:])
                            zps = agps.tile([EMB, P], FP, space="PSUM", tag="zps",
                                            bufs=2)
                            nc.tensor.matmul(zps[:], lhsT=w2sb[:],
                                             rhs=a2dT[:], start=True, stop=True)
                            nc.scalar.activation(ztsb[:, bass.ts(t, P)], zps[:],
                                                 _AF.Identity, bias=b2sb[:, :1])
                if layer == 2:
                    nc.sync.dma_start(ztloc[:], ztsb[:])
                    nc.sync.dma_start(z_out[:], ztsb[:])

        aggregate(g1full, 1)
        nc.gpsimd.collective_compute(
            "AllGather", _OP.bypass, replica_groups=[list(range(NCORES))],
            ins=[g2loc[:].opt()], outs=[g2full[:].opt()])

        aggregate(g2full, 2)
        nc.gpsimd.collective_compute(
            "AllGather", _OP.bypass, replica_groups=[list(range(NCORES))],
            ins=[ztloc[:].opt()], outs=[ztstack[:].opt()])

        # ---- phase 5: decoder od = softplus(z @ z.T + db) -----------
        OCH = 8192            # output buffer width
        DCH = 2048            # psum / ACT chunk width
        with tc.tile_pool(name="p5", bufs=2) as p5, \
             tc.tile_pool(name="p5ps", bufs=2, space="PSUM") as p5ps:
            ztfull = p5.tile([EMB, N], BF, tag="ztfull", bufs=1)
            for d in range(NCORES):
                nc.gpsimd.dma_start(ztfull[:, bass.ts(d, NLOC)],
                                    ztstack[bass.ts(d, EMB), :])
            # this core's own zT rows for the stationary side
            ztmine = p5.tile([EMB, NLOC], BF, tag="ztmine", bufs=1)
            nc.gpsimd.dma_start(ztmine[:], ztloc[:])
            for m in range(NT):
                for h in range(N // OCH):
                    obuf = p5.tile([P, OCH], FP, tag="obuf")
                    tbuf = p5.tile([P, OCH], FP, tag="tbuf", bufs=1)
                    for q in range(OCH // DCH):
                        dps = p5ps.tile([P, DCH], FP, space="PSUM", tag="dps")
                        for s in range(DCH // 512):
                            n0 = h * OCH + q * DCH + s * 512
                            nc.tensor.matmul(
                                dps[:, bass.ts(s, 512)],
                                lhsT=ztmine[:, bass.ts(m, P)],
                                rhs=ztfull[:, n0:n0 + 512],
                                start=True, stop=True)
                        # batch all Exp ops, then one wide Ln: avoids
                        # per-op ACT table thrash (Exp and Ln live in
                        # different default table sets)
                        nc.scalar.activation(tbuf[:, bass.ts(q, DCH)], dps[:],
                                             _AF.Exp, bias=decb[:, :1])
                    nc.scalar.activation(obuf[:], tbuf[:], _AF.Ln, bias=1.0)
                    nc.sync.dma_start(
                        od_out[bass.ts(m, P), bass.ts(h, OCH)], obuf[:])

    nc.compile()
    return nc


_BUILD_CACHE = {}


def kernel(x, edge_index, edge_weight, W1, b1, W2, b2, dec_bias):
    x = np.asarray(x, np.float32)
    edge_index = np.asarray(edge_index, np.int64)
    edge_weight = np.asarray(edge_weight, np.float32)
    W1 = np.asarray(W1, np.float32)
    b1 = np.asarray(b1, np.float32)
    W2 = np.asarray(W2, np.float32)
    b2 = np.asarray(b2, np.float32)
    dec_bias = np.asarray(dec_bias, np.float32)

    per_core, NBT, DMAX = _prep_inputs(x, edge_index[0], edge_index[1],
                                       edge_weight)

    key = (NBT, DMAX)
    if key not in _BUILD_CACHE:
        _BUILD_CACHE[key] = _build(NBT, DMAX)
    nc = _BUILD_CACHE[key]

    in_maps = []
    for c in range(NCORES):
        pc = per_core[c]
        in_maps.append({
            "xT_in": np.ascontiguousarray(
                x[c * NLOC:(c + 1) * NLOC, :].T),
            "w1_in": W1,
            "b1_in": b1.reshape(1, HID),
            "w2_in": W2,
            "b2_in": b2.reshape(EMB, 1),
            "db_in": dec_bias.reshape(1, 1),
            "idx_in": pc["idx"],
            "col_in": pc["colv"],
            "wv_in": pc["wv"],
            "wcsr_in": pc["wcsr"],
        })

    trace = os.environ.get("GCN_TRACE") == "1"
    r = run_bass_kernel_spmd(nc, in_maps, core_ids=list(range(NCORES)),
                             trace=trace)
    if trace and r.exec_time_ns is not None:
        print(f"HW exec time: {r.exec_time_ns} ns")

    od = np.concatenate([r.results[c]["od_out"] for c in range(NCORES)],
                        axis=0)
    z = np.concatenate([r.results[c]["z_out"].T for c in range(NCORES)],
                       axis=0)
    return od, z
